# revision 14
# baseline (speedup 1.0000x reference)
"""Trainium2 Bass kernel for nn_BiLSTMTagger (self-contained).

Strategy: data-parallel over batch across 8 NeuronCores (2 sentences/core).
On device per core: embedding gathers (indirect DMA), 4 BiLSTM layers
(2 shared + 2 SRL). Per layer both directions run fused in one
block-diagonal float32r matmul per timestep; all gate nonlinearities are a
single tanh(0.5x) ACT op via sigmoid(x)=0.5*tanh(x/2)+0.5 with weights
pre-scaled on host (cell state kept as C=2c, H=2h; consumers' weights are
pre-halved to compensate exactly).
Post-LSTM (dep-head MLP branch, role scoring, softmax, CE losses) currently
on host in fp32 (exact), fed by the device-produced BiLSTM sequences.
"""
import sys

sys.path.insert(0, '/opt/trn_rl_repo')
import numpy as np
import concourse.bass as bass
import concourse.tile as tile
from concourse import mybir
from concourse.bass import ts
from concourse.bass_utils import run_bass_kernel_spmd
from concourse.masks import make_identity

F32 = mybir.dt.float32
F32R = mybir.dt.float32r
I32 = mybir.dt.int32

B, L, H = 16, 128, 300
NC = 8
BL = B // NC          # 2 sentences per core
GP = 1280             # padded gate columns: i 0:300 | f 300:600 | o 600:900 | g 900:1200 | pad
K1 = 1024             # layer-1 input features padded (918 -> 8*128)
K2 = 640              # layers 2-4 input features padded (601 -> 5*128)
KH = 640              # recurrence K: Hf 0:300 | Hb 300:600 | pad (5*128)
HB_OFF = 64           # H/Hb tile column offset so Hb transposes land at base-64
NSL = [(0, 512), (512, 512), (1024, 256)]  # gate column slices (psum-bank aligned)


def _split_all_waits(nc, maxw=1):
    """This walrus build allows 1 sync-wait per instruction; hoist extras onto
    same-engine NOPs spliced before the offending instruction."""
    cnt = [0]
    for f in nc.m.functions:
        for bb in f.blocks:
            insts = bb.instructions
            if not any(
                i.sync_info is not None and i.sync_info.on_wait and len(i.sync_info.on_wait) > maxw
                for i in insts
            ):
                continue
            new = []
            for inst in insts:
                si = inst.sync_info
                if si is not None and si.on_wait and len(si.on_wait) > maxw:
                    waits = list(si.on_wait)
                    keep = waits[-maxw:]
                    extra = waits[:-maxw]
                    for j in range(0, len(extra), maxw):
                        cnt[0] += 1
                        nop = mybir.InstNoOp(
                            name=f"I-wsplit-{cnt[0]}",
                            engine=inst.engine,
                            bass_nofuse=True,
                            sync_info=mybir.SyncInfo(on_wait=extra[j:j + maxw], on_update=[]),
                        )
                        nc.register_instruction(nop)
                        new.append(nop)
                    inst.sync_info = mybir.SyncInfo(on_wait=keep, on_update=list(si.on_update or []))
                new.append(inst)
            bb.instructions = new


def _layer_scan(nc, tc, ctx, lname, xp_f, xp_b, whh_rhs, i4, ident, hseq_f, hseq_b,
                sbuf, psum_g, psum_t, stage_pool):
    """Run one BiLSTM layer's 128 timesteps, both directions fused.

    xp_f/xp_b: SBUF [128, 2*GP] token-major input projections (dir f/b),
      batch j at cols j*GP.
    whh_rhs: SBUF [128, 5*GP] f32r recurrent weights in block-diag K layout.
    hseq_f/hseq_b: DRAM [BL, L, H] outputs (true time order, H=2h scale).
    """
    M = 34  # batch-dir rows: f at 0:2, b at 32:34 (PE base-partition rule)
    lhsT = sbuf.tile([128, 5 * M], F32R, tag=f"{lname}_lhsT")
    nc.gpsimd.memset(lhsT[:].bitcast(F32), 0.0)
    ctile = sbuf.tile([M, H], F32, tag=f"{lname}_C")
    nc.gpsimd.memset(ctile[:], 0.0)
    htile = sbuf.tile([M, 388], F32R, tag=f"{lname}_H")
    nc.gpsimd.memset(htile[:].bitcast(F32), 0.0)
    tall = sbuf.tile([M, 1200], F32, tag=f"{lname}_tall")
    u1 = sbuf.tile([M, H], F32, tag=f"{lname}_u1")
    sf = sbuf.tile([M, H], F32, tag=f"{lname}_sf")
    mm = sbuf.tile([M, H], F32, tag=f"{lname}_m")
    thc = sbuf.tile([M, H], F32, tag=f"{lname}_thc")

    for t in range(L):
        tb = L - 1 - t
        stage = stage_pool.tile([4, GP], F32R, tag="stage")
        for j in range(2):
            nc.sync.dma_start(
                out=stage[j:j + 1, :],
                in_=xp_f[t:t + 1, j * GP:(j + 1) * GP].bitcast(F32R))
            nc.sync.dma_start(
                out=stage[2 + j:3 + j, :],
                in_=xp_b[tb:tb + 1, j * GP:(j + 1) * GP].bitcast(F32R))
        gates = psum_g.tile([M, GP], F32, space="PSUM", tag="gates")
        for kc in range(5):
            for (n0, nn) in NSL:
                nc.tensor.matmul(
                    out=gates[:, n0:n0 + nn],
                    lhsT=lhsT[:, M * kc:M * kc + M],
                    rhs=whh_rhs[:, GP * kc + n0:GP * kc + n0 + nn],
                    start=(kc == 0), stop=False,
                )
        for si_, (n0, nn) in enumerate(NSL):
            nc.tensor.matmul(
                out=gates[:, n0:n0 + nn],
                lhsT=i4[:4, :],
                rhs=stage[:, n0:n0 + nn],
                start=False, stop=(si_ == len(NSL) - 1),
            )
        # t_all = tanh(0.5 * gates)  (i,f,o halves give 2*sig-1 inputs; g cols pre-doubled)
        nc.scalar.activation(out=tall[:], in_=gates[:, 0:1200],
                             func=mybir.ActivationFunctionType.Tanh, scale=0.5)
        # u1 = (t_i + 1) * t_g  == 2*sig(i)*tanh(g)
        nc.vector.scalar_tensor_tensor(out=u1[:], in0=tall[:, 0:300], scalar=1.0,
                                       in1=tall[:, 900:1200],
                                       op0=mybir.AluOpType.add, op1=mybir.AluOpType.mult)
        # sf = sig(f)
        nc.vector.tensor_scalar(out=sf[:], in0=tall[:, 300:600], scalar1=0.5, scalar2=0.5,
                                op0=mybir.AluOpType.mult, op1=mybir.AluOpType.add)
        # C = sf*C + u1   (C stored as 2c)
        nc.vector.tensor_tensor(out=mm[:], in0=sf[:], in1=ctile[:], op=mybir.AluOpType.mult)
        nc.vector.tensor_tensor(out=ctile[:], in0=mm[:], in1=u1[:], op=mybir.AluOpType.add)
        # thc = tanh(c) = tanh(C/2)
        nc.scalar.activation(out=thc[:], in_=ctile[:],
                             func=mybir.ActivationFunctionType.Tanh, scale=0.5)
        # H = (t_o + 1) * thc == 2*sig(o)*tanh(c)
        nc.vector.scalar_tensor_tensor(out=htile[:, HB_OFF:HB_OFF + H], in0=tall[:, 600:900],
                                       scalar=1.0, in1=thc[:],
                                       op0=mybir.AluOpType.add, op1=mybir.AluOpType.mult)
        # stream H out to DRAM (true time order)
        nc.sync.dma_start(out=hseq_f[:, t, :],
                          in_=htile[0:2, HB_OFF:HB_OFF + H].bitcast(F32))
        nc.sync.dma_start(out=hseq_b[:, tb, :],
                          in_=htile[32:34, HB_OFF:HB_OFF + H].bitcast(F32))
        # transposes: Hf pieces -> lhsT chunks 0,1,2a ; Hb pieces -> 2b,3,4
        # Hf slices at cols HB_OFF+[0:128,128:256,256:300]; Hb padded slices at [0:128,128:256,256:384]
        pieces = [
            (htile[0:2, HB_OFF:HB_OFF + 128], 0, 0, 128, 0),       # Hf[0:128]
            (htile[0:2, HB_OFF + 128:HB_OFF + 256], 1, 0, 128, 0),
            (htile[0:2, HB_OFF + 256:HB_OFF + 300], 2, 0, 44, 0),  # Hf[256:300]
            (htile[32:34, 0:128], 2, 64, 128, 32),                 # Hb[0:64] -> rows 64:128
            (htile[32:34, 128:256], 3, 0, 128, 32),                # Hb[64:192]
            (htile[32:34, 256:384], 4, 0, 128, 32),                # Hb[192:300]+pad
        ]
        for (piece, chunk, r0, r1, c0) in pieces:
            tp = psum_t.tile([128, 2], F32, space="PSUM", tag="tp")
            nc.tensor.transpose(out=tp[:piece.shape[1], :], in_=piece.bitcast(F32),
                                identity=ident[c0:c0 + 2, c0:c0 + 2])
            nc.vector.tensor_copy(out=lhsT[r0:r1, M * chunk + c0:M * chunk + c0 + 2],
                                  in_=tp[r0:r1, :])


def _build_xT_from_hseq(nc, tc, hf_t, hb_t, xT, ident, psum_x, ones_t):
    """hf_t [128,304] (Hf at 0:300), hb_t [128,384] (Hb at cols 44:344) ->
    xT [128, 5*128] f32r: rows = features [Hf 0:300 | Hb 300:600 | ones 600 | pad]."""
    pieces = [
        (hf_t[:, 0:128], 0, 0, 128),
        (hf_t[:, 128:256], 1, 0, 128),
        (hf_t[:, 256:300], 2, 0, 44),
        (hb_t[:, 0:128], 2, 64, 128),      # Hb[0:64] -> feature rows 320:384
        (hb_t[:, 128:256], 3, 0, 128),     # Hb[64:192]
        (hb_t[:, 256:384], 4, 0, 128),     # Hb[192:300]+pad
    ]
    for (src, chunk, r0, r1) in pieces:
        tp = psum_x.tile([128, 128], F32, space="PSUM", tag="tp")
        nc.tensor.transpose(out=tp[:src.shape[1], :], in_=src, identity=ident[:])  # [128,*] full K
        nc.vector.tensor_copy(out=xT[r0:r1, ts(chunk, 128)], in_=tp[r0:r1, :])
    # ones row at feature 620 = chunk 4 row 108 (DMA: engines can't address base 108)
    nc.sync.dma_start(out=xT[108:109, ts(4, 128)], in_=ones_t[:, 0:128].bitcast(F32R))


def _proj(nc, xT, w_rhs, xp_out, col0, kcs, psum_p, sbuf_unused):
    """xp_out[:, col0:col0+GP] = (xT.T @ w_rhs) ; xT [128, kcs*128] f32r,
    w_rhs [128, kcs*GP] f32r, out token-major [128, GP] fp32."""
    pt = psum_p.tile([128, GP], F32, space="PSUM", tag="gates")
    for (n0, nn) in NSL:
        for kc in range(kcs):
            nc.tensor.matmul(
                out=pt[:, n0:n0 + nn],
                lhsT=xT[:, ts(kc, 128)],
                rhs=w_rhs[:, GP * kc + n0:GP * kc + n0 + nn],
                start=(kc == 0), stop=(kc == kcs - 1),
            )
    nc.vector.tensor_copy(out=xp_out[:, col0:col0 + GP], in_=pt[:])


_PROGRAM_CACHE = {}


def _build_program():
    if "nc" in _PROGRAM_CACHE:
        return _PROGRAM_CACHE["nc"]
    nc = bass.Bass()
    dp = nc.declare_dram_parameter
    # ---- inputs ----
    emb_word = dp("emb_word", [50000, 300], F32, isOutput=False)
    emb_fixed = dp("emb_fixed", [50000, 300], F32, isOutput=False)
    emb_pos = dp("emb_pos", [60, 16], F32, isOutput=False)
    emb_lemma = dp("emb_lemma", [5000, 300], F32, isOutput=False)
    sent_i = dp("sent_i", [BL, 128], I32, isOutput=False)
    psent_i = dp("psent_i", [BL, 128], I32, isOutput=False)
    pos_i = dp("pos_i", [BL, 128], I32, isOutput=False)
    lemma_i = dp("lemma_i", [BL, 128], I32, isOutput=False)
    region = dp("region", [BL, 128], F32, isOutput=False)
    i4m = dp("i4m", [4, 34], F32, isOutput=False)
    onesrow = dp("onesrow", [1, 128], F32, isOutput=False)
    wih = {}
    for l in range(1, 5):
        kc = 8 if l == 1 else 5
        for d in "fb":
            wih[(l, d)] = dp(f"wih{l}{d}", [128, kc * GP], F32, isOutput=False)
    whh = {l: dp(f"whh{l}", [128, 5 * GP], F32, isOutput=False) for l in range(1, 5)}
    # ---- outputs: the four BiLSTM layer sequences we need downstream ----
    hseq = {}
    for l, n in [(2, "bfe"), (4, "h3")]:
        for d in "fb":
            hseq[(l, d)] = dp(f"hseq_{n}_{d}", [BL, L, H], F32, isOutput=True)
    for l in [1, 3]:
        for d in "fb":
            hseq[(l, d)] = nc.dram_tensor(f"hseq_l{l}_{d}", [BL, L, H], F32)

    with tile.TileContext(nc) as tc:
        import contextlib
        ctx = contextlib.ExitStack()
        with ctx:
            sbuf = ctx.enter_context(tc.tile_pool(name="sbuf", bufs=1))
            psum_g = ctx.enter_context(tc.tile_pool(name="psg", bufs=2, space="PSUM"))
            psum_t = ctx.enter_context(tc.tile_pool(name="pst", bufs=2, space="PSUM"))
            stage_pool = ctx.enter_context(tc.tile_pool(name="stage", bufs=3))
            wpool = ctx.enter_context(tc.tile_pool(name="wpool", bufs=1))

            ident = sbuf.tile([128, 128], F32, tag="ident")
            make_identity(nc, ident[:])
            ones_t = sbuf.tile([1, 128], F32, tag="ones_t")
            nc.sync.dma_start(out=ones_t[:], in_=onesrow[:])
            i4 = sbuf.tile([4, 34], F32R, tag="i4")
            nc.sync.dma_start(out=i4[:], in_=i4m[:].bitcast(F32R))

            # ---- embeddings gather -> x [128, 1024] per b; transpose -> x1T ----
            x1T = [sbuf.tile([128, 8 * 128], F32R, name=f"x1T{b}", tag=f"x1T{b}") for b in range(BL)]
            for b in range(BL):
                xt = sbuf.tile([128, K1], F32, tag="xgath")
                nc.gpsimd.memset(xt[:], 0.0)
                for (idx_t, table, c0, w) in [
                    (sent_i, emb_word, 0, 300),
                    (psent_i, emb_fixed, 300, 300),
                    (pos_i, emb_pos, 600, 16),
                    (lemma_i, emb_lemma, 616, 300),
                ]:
                    it = sbuf.tile([128, 1], I32, tag="idx")
                    nc.sync.dma_start(out=it[:, 0:1], in_=idx_t[b, :, None])
                    nc.gpsimd.indirect_dma_start(
                        out=xt[:, c0:c0 + w], out_offset=None, in_=table[:],
                        in_offset=bass.IndirectOffsetOnAxis(ap=it[:, :1], axis=0),
                    )
                nc.sync.dma_start(out=xt[:, 916:917], in_=region[b, :, None])
                nc.gpsimd.memset(xt[:, 917:918], 1.0)
                for kc in range(8):
                    tp = psum_t.tile([128, 128], F32, space="PSUM", tag="tp")
                    nc.tensor.transpose(out=tp[:], in_=xt[:, ts(kc, 128)], identity=ident[:])
                    nc.vector.tensor_copy(out=x1T[b][:, ts(kc, 128)], in_=tp[:])

            # ---- per-layer: projections then scan ----
            hf_t = [sbuf.tile([128, 304], F32, name=f"hf{b}", tag=f"hf{b}") for b in range(BL)]
            hb_t = [sbuf.tile([128, 384], F32, name=f"hb{b}", tag=f"hb{b}") for b in range(BL)]
            x2T = [sbuf.tile([128, 5 * 128], F32R, name=f"x2T{b}", tag=f"x2T{b}") for b in range(BL)]
            for b in range(BL):
                nc.gpsimd.memset(hb_t[b][:], 0.0)
                nc.gpsimd.memset(x2T[b][:].bitcast(F32), 0.0)

            for l in range(1, 5):
                kc = 8 if l == 1 else 5
                xp_f = sbuf.tile([128, 2 * GP], F32, tag="xp_f")
                xp_b = sbuf.tile([128, 2 * GP], F32, tag="xp_b")
                wtile = wpool.tile([128, kc * GP], F32R, tag="wtile")
                for d, xp in (("f", xp_f), ("b", xp_b)):
                    nc.sync.dma_start(out=wtile[:], in_=wih[(l, d)][:].bitcast(F32R))
                    for b in range(BL):
                        if l == 1:
                            xT = x1T[b]
                        else:
                            xT = x2T[b]
                        _proj(nc, xT, wtile, xp, b * GP, kc, psum_g, sbuf)
                whh_rhs = wpool.tile([128, 5 * GP], F32R, tag="whh_rhs")
                nc.sync.dma_start(out=whh_rhs[:], in_=whh[l][:].bitcast(F32R))
                _layer_scan(nc, tc, ctx, f"l{l}", xp_f, xp_b, whh_rhs, i4, ident,
                            hseq[(l, "f")], hseq[(l, "b")], sbuf, psum_g, psum_t, stage_pool)
                if l < 4:
                    for b in range(BL):
                        nc.sync.dma_start(out=hf_t[b][:, 0:300], in_=hseq[(l, "f")][b, :, :])
                        nc.sync.dma_start(out=hb_t[b][:, HB_OFF:HB_OFF + 300],  # Hb at cols 64:364
                                          in_=hseq[(l, "b")][b, :, :])
                        _build_xT_from_hseq(nc, tc, hf_t[b], hb_t[b], x2T[b], ident, psum_t, ones_t)

    _split_all_waits(nc)
    _PROGRAM_CACHE["nc"] = nc
    return nc


def _prep_lstm_weights(params):
    """Reorder gates i,f,g,o -> i,f,o,g; double g rows (tanh(0.5*2x)=tanh x);
    scale input weights of consumers of H=2h by 0.5; fold bias as ones-feature."""
    def reorder(w):
        i, f, g, o = np.split(w, 4, axis=0)
        return np.concatenate([i, f, o, 2.0 * g], axis=0)

    out = {}
    layers = list(params['lstm_share']) + list(params['lstm_srl'])
    for li, dirs in enumerate(layers, start=1):
        kc = 8 if li == 1 else 5
        whh_blk = np.zeros((5 * 128, GP), np.float32)
        for di, dname in enumerate("fb"):
            p = dirs[di]
            wih = reorder(np.asarray(p['wih'], np.float32))      # [1200, din]
            bb_ = reorder(np.asarray(p['b'], np.float32)[:, None])[:, 0]
            whh_ = reorder(np.asarray(p['whh'], np.float32))     # [1200, 300]
            din = wih.shape[1]
            if li > 1:
                wih = wih * 0.5           # input is H=2h
            whh_ = whh_ * 0.5             # recurrent input is H=2h
            wr = np.zeros((kc * 128, GP), np.float32)
            if li == 1:
                wr[:din, :1200] = wih.T
                wr[917, :1200] = bb_      # ones feature at col 917
            else:
                wr[0:300, :1200] = wih.T[0:300]      # Hf features
                wr[320:620, :1200] = wih.T[300:600]  # Hb features
                wr[620, :1200] = bb_                 # ones feature
            out[f"wih{li}{dname}"] = np.ascontiguousarray(
                wr.reshape(kc, 128, GP).transpose(1, 0, 2).reshape(128, kc * GP))
            r0 = 0 if dname == "f" else 320
            whh_blk[r0:r0 + 300, :1200] = whh_.T
        out[f"whh{li}"] = np.ascontiguousarray(
            whh_blk.reshape(5, 128, GP).transpose(1, 0, 2).reshape(128, 5 * GP))
    return out


def _host_post(bf_e, h3, inputs, params):
    """Exact fp32 post-LSTM math on host. bf_e/h3: [B, L, 600] true scale."""
    p = params
    _BIG = 10.0 ** 6.0
    dep_heads = np.asarray(inputs['dep_heads'])
    idx = np.clip(dep_heads - 1, 0, L - 1)
    gathered = np.take_along_axis(bf_e, idx[:, :, None], axis=1)
    concat_embeds = np.where((dep_heads > 0)[:, :, None], gathered, 0.0).astype(np.float32)

    def lin(x, pp):
        return x @ np.asarray(pp['w'], np.float32).T + np.asarray(pp['b'], np.float32)

    dep_tag_space = lin(np.tanh(lin(bf_e, p['h2t_M']) + lin(concat_embeds, p['h2t_H'])),
                        p['mlp']).reshape(B * L, -1)
    pred = h3[np.arange(B), np.asarray(inputs['target_idx_in'])]
    hs = np.concatenate([h3, np.broadcast_to(pred[:, None, :], h3.shape)], axis=-1)
    role = np.concatenate([np.asarray(p['role_emb'], np.float32)[np.asarray(inputs['local_roles_voc'])],
                           np.asarray(p['frame_emb'], np.float32)[np.asarray(inputs['frames'])]],
                          axis=-1)
    mapped = np.maximum(lin(role, p['role_map']), 0.0)
    tag_space = np.einsum('blh,brh->blr', hs, mapped)
    sub = (np.asarray(inputs['local_roles_mask'], np.float32) - 1.0) * _BIG
    tag_space = tag_space + sub[:, None, :]
    tag2 = tag_space.reshape(B * L, -1).astype(np.float32)

    def logsoftmax(x):
        m = x.max(axis=1, keepdims=True)
        e = np.exp(x - m)
        return (x - m) - np.log(e.sum(axis=1, keepdims=True))

    SRLprobs = np.exp(logsoftmax(tag2))

    def ce_ignore0(logits, tgt):
        logp = logsoftmax(logits)
        nll = -logp[np.arange(logits.shape[0]), tgt]
        m = (tgt != 0).astype(np.float32)
        return (nll * m).sum() / max(m.sum(), 1.0)

    dep_labels = np.argmax(dep_tag_space, axis=1)
    gold = np.asarray(inputs['dep_tags']).reshape(-1)
    all_l = np.float32((gold != 0).sum())
    wrong_l = np.float32(((dep_labels != gold) & (gold != 0)).sum())
    SRLloss = np.float32(ce_ignore0(tag2, np.asarray(inputs['targets']).reshape(-1)))
    DEPloss = np.float32(ce_ignore0(dep_tag_space, gold))
    return SRLloss, DEPloss, np.float32(SRLloss + DEPloss), SRLprobs.astype(np.float32), wrong_l, all_l


def kernel(**inputs):
    params = inputs['params']
    nc = _build_program()
    wmap = _prep_lstm_weights(params)

    def np32(x):
        return np.ascontiguousarray(np.asarray(x, np.float32))

    tables = {
        "emb_word": np32(params['word_emb']),
        "emb_fixed": np32(params['word_fixed']),
        "emb_pos": np32(params['pos_emb']),
        "emb_lemma": np32(params['p_lemma_emb']),
    }
    in_maps = []
    for c in range(NC):
        sl = slice(c * BL, (c + 1) * BL)
        m = dict(tables)
        m.update({k: v for k, v in wmap.items()})
        m["sent_i"] = np.ascontiguousarray(np.asarray(inputs['sentence'][sl], np.int32))
        m["psent_i"] = np.ascontiguousarray(np.asarray(inputs['p_sentence'][sl], np.int32))
        m["pos_i"] = np.ascontiguousarray(np.asarray(inputs['pos_tags'][sl], np.int32))
        m["lemma_i"] = np.ascontiguousarray(np.asarray(inputs['sent_pred_lemmas_idx'][sl], np.int32))
        m["region"] = np32(inputs['region_marks'][sl])
        i4v = np.zeros((4, 34), np.float32)
        for j, col in enumerate([0, 1, 32, 33]):
            i4v[j, col] = 1.0
        m["i4m"] = i4v
        m["onesrow"] = np.ones((1, 128), np.float32)
        in_maps.append(m)

    import os
    trace = bool(os.environ.get("BASS_TRACE"))
    res = run_bass_kernel_spmd(nc, in_maps, core_ids=list(range(NC)), trace=trace)
    globals()["LAST_RESULTS"] = res
    bf_e = np.zeros((B, L, 2 * H), np.float32)
    h3 = np.zeros((B, L, 2 * H), np.float32)
    for c in range(NC):
        r = res.results[c]
        sl = slice(c * BL, (c + 1) * BL)
        bf_e[sl, :, 0:H] = r["hseq_bfe_f"] * 0.5   # H=2h -> true h
        bf_e[sl, :, H:2 * H] = r["hseq_bfe_b"] * 0.5
        h3[sl, :, 0:H] = r["hseq_h3_f"] * 0.5
        h3[sl, :, H:2 * H] = r["hseq_h3_b"] * 0.5
    return _host_post(bf_e, h3, inputs, params)


# revision 17
# speedup vs baseline: 1.2739x; 1.2739x over previous
"""Trainium2 Bass kernel for nn_BiLSTMTagger (self-contained).

Strategy: data-parallel over batch across 8 NeuronCores (2 sentences/core).
On device per core: embedding gathers (indirect DMA), 4 BiLSTM layers
(2 shared + 2 SRL). Per layer both directions run fused in one
block-diagonal float32r matmul per timestep; all gate nonlinearities are a
single tanh(0.5x) ACT op via sigmoid(x)=0.5*tanh(x/2)+0.5 with weights
pre-scaled on host (cell state kept as C=2c, H=2h; consumers' weights are
pre-halved to compensate exactly).
Post-LSTM (dep-head MLP branch, role scoring, softmax, CE losses) currently
on host in fp32 (exact), fed by the device-produced BiLSTM sequences.
"""
import sys

sys.path.insert(0, '/opt/trn_rl_repo')
import numpy as np
import concourse.bass as bass
import concourse.tile as tile
from concourse import mybir
from concourse.bass import ts
from concourse.bass_utils import run_bass_kernel_spmd
from concourse.masks import make_identity

F32 = mybir.dt.float32
F32R = mybir.dt.float32r
I32 = mybir.dt.int32

B, L, H = 16, 128, 300
NC = 8
BL = B // NC          # 2 sentences per core
GP = 1280             # padded gate columns: i 0:300 | f 300:600 | o 600:900 | g 900:1200 | pad
K1 = 1024             # layer-1 input features padded (918 -> 8*128)
K2 = 640              # layers 2-4 input features padded (601 -> 5*128)
KH = 640              # recurrence K: Hf 0:300 | Hb 300:600 | pad (5*128)
HB_OFF = 64           # H/Hb tile column offset so Hb transposes land at base-64
NSL = [(0, 512), (512, 512), (1024, 256)]  # gate column slices (psum-bank aligned)


def _split_all_waits(nc, maxw=1):
    """This walrus build allows 1 sync-wait per instruction; hoist extras onto
    same-engine NOPs spliced before the offending instruction."""
    cnt = [0]
    for f in nc.m.functions:
        for bb in f.blocks:
            insts = bb.instructions
            if not any(
                i.sync_info is not None and i.sync_info.on_wait and len(i.sync_info.on_wait) > maxw
                for i in insts
            ):
                continue
            new = []
            for inst in insts:
                si = inst.sync_info
                if si is not None and si.on_wait and len(si.on_wait) > maxw:
                    waits = list(si.on_wait)
                    keep = waits[-maxw:]
                    extra = waits[:-maxw]
                    for j in range(0, len(extra), maxw):
                        cnt[0] += 1
                        nop = mybir.InstNoOp(
                            name=f"I-wsplit-{cnt[0]}",
                            engine=inst.engine,
                            bass_nofuse=True,
                            sync_info=mybir.SyncInfo(on_wait=extra[j:j + maxw], on_update=[]),
                        )
                        nc.register_instruction(nop)
                        new.append(nop)
                    inst.sync_info = mybir.SyncInfo(on_wait=keep, on_update=list(si.on_update or []))
                new.append(inst)
            bb.instructions = new


def _layer_scan(nc, tc, ctx, lname, xp_f, xp_b, whh_rhs_f, whh_rhs_b, i2r, ident,
                hseq_f, hseq_b, sbuf, psum_g, psum_t, stage_pool):
    """One BiLSTM layer, 128 steps. Two independent chains (dir f and b), each
    batch-major [2, *]; they pipeline across PE/ACT/DVE.

    xp_d: SBUF [128, 2*GP] token-major input projections, batch j at cols j*GP.
    whh_rhs_d: SBUF [128, 3*GP] f32r recurrent weights (K = Hf 0:300, 3 chunks).
    hseq_d: DRAM [BL, L, H] outputs (true time order, H=2h scale).
    """
    chains = []
    for d, xp, whh_rhs, hseq in (("f", xp_f, whh_rhs_f, hseq_f),
                                 ("b", xp_b, whh_rhs_b, hseq_b)):
        lhsT = sbuf.tile([128, 3 * 2], F32R, name=f"lhsT{d}", tag=f"scan_lhsT{d}")
        nc.gpsimd.memset(lhsT[:].bitcast(F32), 0.0)
        ctile = sbuf.tile([2, H], F32, name=f"C{d}", tag=f"scan_C{d}")
        nc.gpsimd.memset(ctile[:], 0.0)
        htile = sbuf.tile([2, H], F32R, name=f"H{d}", tag=f"scan_H{d}")
        nc.gpsimd.memset(htile[:].bitcast(F32), 0.0)
        tall = sbuf.tile([2, 1200], F32, name=f"tall{d}", tag=f"scan_tall{d}")
        u1 = sbuf.tile([2, H], F32, name=f"u1{d}", tag=f"scan_u1{d}")
        sf = sbuf.tile([2, H], F32, name=f"sf{d}", tag=f"scan_sf{d}")
        mm = sbuf.tile([2, H], F32, name=f"mm{d}", tag=f"scan_m{d}")
        thc = sbuf.tile([2, H], F32, name=f"thc{d}", tag=f"scan_thc{d}")
        chains.append((d, xp, whh_rhs, hseq, lhsT, ctile, htile, tall, u1, sf, mm, thc))

    for t in range(L):
        for (d, xp, whh_rhs, hseq, lhsT, ctile, htile, tall, u1, sf, mm, thc) in chains:
            tt = t if d == "f" else L - 1 - t
            stage = stage_pool.tile([2, GP], F32R, tag=f"stage{d}")
            for j in range(2):
                nc.sync.dma_start(out=stage[j:j + 1, :],
                                  in_=xp[tt:tt + 1, j * GP:(j + 1) * GP].bitcast(F32R))
            gates = psum_g.tile([2, GP], F32, space="PSUM", tag=f"gates{d}")
            for kc in range(3):
                for (n0, nn) in NSL:
                    nc.tensor.matmul(
                        out=gates[:, n0:n0 + nn],
                        lhsT=lhsT[:, 2 * kc:2 * kc + 2],
                        rhs=whh_rhs[:, GP * kc + n0:GP * kc + n0 + nn],
                        start=(kc == 0), stop=False,
                    )
            for si_, (n0, nn) in enumerate(NSL):
                nc.tensor.matmul(
                    out=gates[:, n0:n0 + nn],
                    lhsT=i2r[:2, :2],
                    rhs=stage[:, n0:n0 + nn],
                    start=False, stop=(si_ == len(NSL) - 1),
                )
            # t_all = tanh(0.5*gates); u1 = (t_i+1)*t_g; sf = sig(f)
            nc.scalar.activation(out=tall[:], in_=gates[:, 0:1200],
                                 func=mybir.ActivationFunctionType.Tanh, scale=0.5)
            nc.vector.scalar_tensor_tensor(out=u1[:], in0=tall[:, 0:300], scalar=1.0,
                                           in1=tall[:, 900:1200],
                                           op0=mybir.AluOpType.add, op1=mybir.AluOpType.mult)
            nc.vector.tensor_scalar(out=sf[:], in0=tall[:, 300:600], scalar1=0.5, scalar2=0.5,
                                    op0=mybir.AluOpType.mult, op1=mybir.AluOpType.add)
            nc.vector.tensor_tensor(out=mm[:], in0=sf[:], in1=ctile[:], op=mybir.AluOpType.mult)
            nc.vector.tensor_tensor(out=ctile[:], in0=mm[:], in1=u1[:], op=mybir.AluOpType.add)
            nc.scalar.activation(out=thc[:], in_=ctile[:],
                                 func=mybir.ActivationFunctionType.Tanh, scale=0.5)
            nc.vector.scalar_tensor_tensor(out=htile[:], in0=tall[:, 600:900],
                                           scalar=1.0, in1=thc[:],
                                           op0=mybir.AluOpType.add, op1=mybir.AluOpType.mult)
            nc.sync.dma_start(out=hseq[:, tt, :], in_=htile[:].bitcast(F32))
            for kc, (c0, cw) in enumerate([(0, 128), (128, 128), (256, 44)]):
                tp = psum_t.tile([128, 2], F32, space="PSUM", tag="tp")
                nc.tensor.transpose(out=tp[:cw, :], in_=htile[:, c0:c0 + cw].bitcast(F32),
                                    identity=ident[:2, :2])
                nc.vector.tensor_copy(out=lhsT[0:cw, 2 * kc:2 * kc + 2], in_=tp[0:cw, :])


def _build_xT_from_hseq(nc, tc, hf_t, hb_t, xT, ident, psum_x, ones_t):
    """hf_t [128,304] (Hf at 0:300), hb_t [128,384] (Hb at cols 44:344) ->
    xT [128, 5*128] f32r: rows = features [Hf 0:300 | Hb 300:600 | ones 600 | pad]."""
    pieces = [
        (hf_t[:, 0:128], 0, 0, 128),
        (hf_t[:, 128:256], 1, 0, 128),
        (hf_t[:, 256:300], 2, 0, 44),
        (hb_t[:, 0:128], 2, 64, 128),      # Hb[0:64] -> feature rows 320:384
        (hb_t[:, 128:256], 3, 0, 128),     # Hb[64:192]
        (hb_t[:, 256:384], 4, 0, 128),     # Hb[192:300]+pad
    ]
    for (src, chunk, r0, r1) in pieces:
        tp = psum_x.tile([128, 128], F32, space="PSUM", tag="tp")
        nc.tensor.transpose(out=tp[:src.shape[1], :], in_=src, identity=ident[:])  # [128,*] full K
        nc.vector.tensor_copy(out=xT[r0:r1, ts(chunk, 128)], in_=tp[r0:r1, :])
    # ones row at feature 620 = chunk 4 row 108 (DMA: engines can't address base 108)
    nc.sync.dma_start(out=xT[108:109, ts(4, 128)], in_=ones_t[:, 0:128].bitcast(F32R))


def _proj(nc, xT, w_rhs, xp_out, col0, kcs, psum_p, sbuf_unused):
    """xp_out[:, col0:col0+GP] = (xT.T @ w_rhs) ; xT [128, kcs*128] f32r,
    w_rhs [128, kcs*GP] f32r, out token-major [128, GP] fp32."""
    pt = psum_p.tile([128, GP], F32, space="PSUM", tag="gatesf")
    for (n0, nn) in NSL:
        for kc in range(kcs):
            nc.tensor.matmul(
                out=pt[:, n0:n0 + nn],
                lhsT=xT[:, ts(kc, 128)],
                rhs=w_rhs[:, GP * kc + n0:GP * kc + n0 + nn],
                start=(kc == 0), stop=(kc == kcs - 1),
            )
    nc.vector.tensor_copy(out=xp_out[:, col0:col0 + GP], in_=pt[:])


_PROGRAM_CACHE = {}


def _build_program():
    if "nc" in _PROGRAM_CACHE:
        return _PROGRAM_CACHE["nc"]
    nc = bass.Bass()
    dp = nc.declare_dram_parameter
    # ---- inputs ----
    emb_word = dp("emb_word", [50000, 300], F32, isOutput=False)
    emb_fixed = dp("emb_fixed", [50000, 300], F32, isOutput=False)
    emb_pos = dp("emb_pos", [60, 16], F32, isOutput=False)
    emb_lemma = dp("emb_lemma", [5000, 300], F32, isOutput=False)
    sent_i = dp("sent_i", [BL, 128], I32, isOutput=False)
    psent_i = dp("psent_i", [BL, 128], I32, isOutput=False)
    pos_i = dp("pos_i", [BL, 128], I32, isOutput=False)
    lemma_i = dp("lemma_i", [BL, 128], I32, isOutput=False)
    region = dp("region", [BL, 128], F32, isOutput=False)
    i4m = dp("i4m", [4, 34], F32, isOutput=False)
    onesrow = dp("onesrow", [1, 128], F32, isOutput=False)
    wih = {}
    for l in range(1, 5):
        kc = 8 if l == 1 else 5
        for d in "fb":
            wih[(l, d)] = dp(f"wih{l}{d}", [128, kc * GP], F32, isOutput=False)
    whh = {(l, d): dp(f"whh{l}{d}", [128, 3 * GP], F32, isOutput=False)
           for l in range(1, 5) for d in "fb"}
    # ---- outputs: the four BiLSTM layer sequences we need downstream ----
    hseq = {}
    for l, n in [(2, "bfe"), (4, "h3")]:
        for d in "fb":
            hseq[(l, d)] = dp(f"hseq_{n}_{d}", [BL, L, H], F32, isOutput=True)
    for l in [1, 3]:
        for d in "fb":
            hseq[(l, d)] = nc.dram_tensor(f"hseq_l{l}_{d}", [BL, L, H], F32)

    with tile.TileContext(nc) as tc:
        import contextlib
        ctx = contextlib.ExitStack()
        with ctx:
            sbuf = ctx.enter_context(tc.tile_pool(name="sbuf", bufs=1))
            psum_g = ctx.enter_context(tc.tile_pool(name="psg", bufs=1, space="PSUM"))
            psum_t = ctx.enter_context(tc.tile_pool(name="pst", bufs=2, space="PSUM"))
            stage_pool = ctx.enter_context(tc.tile_pool(name="stage", bufs=3))
            wpool = ctx.enter_context(tc.tile_pool(name="wpool", bufs=1))

            ident = sbuf.tile([128, 128], F32, tag="ident")
            make_identity(nc, ident[:])
            ones_t = sbuf.tile([1, 128], F32, tag="ones_t")
            nc.sync.dma_start(out=ones_t[:], in_=onesrow[:])
            identr = sbuf.tile([128, 128], F32R, tag="identr")
            nc.vector.tensor_copy(out=identr[:], in_=ident[:])

            # ---- embeddings gather -> x [128, 1024] per b; transpose -> x1T ----
            x1T = [sbuf.tile([128, 8 * 128], F32R, name=f"x1T{b}", tag=f"x1T{b}") for b in range(BL)]
            for b in range(BL):
                xt = sbuf.tile([128, K1], F32, tag="xgath")
                nc.gpsimd.memset(xt[:], 0.0)
                for (idx_t, table, c0, w) in [
                    (sent_i, emb_word, 0, 300),
                    (psent_i, emb_fixed, 300, 300),
                    (pos_i, emb_pos, 600, 16),
                    (lemma_i, emb_lemma, 616, 300),
                ]:
                    it = sbuf.tile([128, 1], I32, tag="idx")
                    nc.sync.dma_start(out=it[:, 0:1], in_=idx_t[b, :, None])
                    nc.gpsimd.indirect_dma_start(
                        out=xt[:, c0:c0 + w], out_offset=None, in_=table[:],
                        in_offset=bass.IndirectOffsetOnAxis(ap=it[:, :1], axis=0),
                    )
                nc.sync.dma_start(out=xt[:, 916:917], in_=region[b, :, None])
                nc.gpsimd.memset(xt[:, 917:918], 1.0)
                for kc in range(8):
                    tp = psum_t.tile([128, 128], F32, space="PSUM", tag="tp")
                    nc.tensor.transpose(out=tp[:], in_=xt[:, ts(kc, 128)], identity=ident[:])
                    nc.vector.tensor_copy(out=x1T[b][:, ts(kc, 128)], in_=tp[:])

            # ---- per-layer: projections then scan ----
            hf_t = [sbuf.tile([128, 304], F32, name=f"hf{b}", tag=f"hf{b}") for b in range(BL)]
            hb_t = [sbuf.tile([128, 384], F32, name=f"hb{b}", tag=f"hb{b}") for b in range(BL)]
            x2T = [sbuf.tile([128, 5 * 128], F32R, name=f"x2T{b}", tag=f"x2T{b}") for b in range(BL)]
            for b in range(BL):
                nc.gpsimd.memset(hb_t[b][:], 0.0)
                nc.gpsimd.memset(x2T[b][:].bitcast(F32), 0.0)

            for l in range(1, 5):
                kc = 8 if l == 1 else 5
                xp_f = sbuf.tile([128, 2 * GP], F32, tag="xp_f")
                xp_b = sbuf.tile([128, 2 * GP], F32, tag="xp_b")
                wtile = wpool.tile([128, kc * GP], F32R, tag="wtile")
                for d, xp in (("f", xp_f), ("b", xp_b)):
                    nc.sync.dma_start(out=wtile[:], in_=wih[(l, d)][:].bitcast(F32R))
                    for b in range(BL):
                        if l == 1:
                            xT = x1T[b]
                        else:
                            xT = x2T[b]
                        _proj(nc, xT, wtile, xp, b * GP, kc, psum_g, sbuf)
                whh_rhs_f = wpool.tile([128, 3 * GP], F32R, tag="whh_rhs_f")
                nc.sync.dma_start(out=whh_rhs_f[:], in_=whh[(l, "f")][:].bitcast(F32R))
                whh_rhs_b = wpool.tile([128, 3 * GP], F32R, tag="whh_rhs_b")
                nc.sync.dma_start(out=whh_rhs_b[:], in_=whh[(l, "b")][:].bitcast(F32R))
                _layer_scan(nc, tc, ctx, f"l{l}", xp_f, xp_b, whh_rhs_f, whh_rhs_b,
                            identr, ident,
                            hseq[(l, "f")], hseq[(l, "b")], sbuf, psum_g, psum_t, stage_pool)
                if l < 4:
                    for b in range(BL):
                        nc.sync.dma_start(out=hf_t[b][:, 0:300], in_=hseq[(l, "f")][b, :, :])
                        nc.sync.dma_start(out=hb_t[b][:, HB_OFF:HB_OFF + 300],  # Hb at cols 64:364
                                          in_=hseq[(l, "b")][b, :, :])
                        _build_xT_from_hseq(nc, tc, hf_t[b], hb_t[b], x2T[b], ident, psum_t, ones_t)

    _split_all_waits(nc)
    _PROGRAM_CACHE["nc"] = nc
    return nc


def _prep_lstm_weights(params):
    """Reorder gates i,f,g,o -> i,f,o,g; double g rows (tanh(0.5*2x)=tanh x);
    scale input weights of consumers of H=2h by 0.5; fold bias as ones-feature."""
    def reorder(w):
        i, f, g, o = np.split(w, 4, axis=0)
        return np.concatenate([i, f, o, 2.0 * g], axis=0)

    out = {}
    layers = list(params['lstm_share']) + list(params['lstm_srl'])
    for li, dirs in enumerate(layers, start=1):
        kc = 8 if li == 1 else 5
        for di, dname in enumerate("fb"):
            p = dirs[di]
            wih = reorder(np.asarray(p['wih'], np.float32))      # [1200, din]
            bb_ = reorder(np.asarray(p['b'], np.float32)[:, None])[:, 0]
            whh_ = reorder(np.asarray(p['whh'], np.float32))     # [1200, 300]
            din = wih.shape[1]
            if li > 1:
                wih = wih * 0.5           # input is H=2h
            whh_ = whh_ * 0.5             # recurrent input is H=2h
            wr = np.zeros((kc * 128, GP), np.float32)
            if li == 1:
                wr[:din, :1200] = wih.T
                wr[917, :1200] = bb_      # ones feature at col 917
            else:
                wr[0:300, :1200] = wih.T[0:300]      # Hf features
                wr[320:620, :1200] = wih.T[300:600]  # Hb features
                wr[620, :1200] = bb_                 # ones feature
            out[f"wih{li}{dname}"] = np.ascontiguousarray(
                wr.reshape(kc, 128, GP).transpose(1, 0, 2).reshape(128, kc * GP))
            whh_blk = np.zeros((3 * 128, GP), np.float32)
            whh_blk[0:300, :1200] = whh_.T
            out[f"whh{li}{dname}"] = np.ascontiguousarray(
                whh_blk.reshape(3, 128, GP).transpose(1, 0, 2).reshape(128, 3 * GP))
    return out


def _host_post(bf_e, h3, inputs, params):
    """Exact fp32 post-LSTM math on host. bf_e/h3: [B, L, 600] true scale."""
    p = params
    _BIG = 10.0 ** 6.0
    dep_heads = np.asarray(inputs['dep_heads'])
    idx = np.clip(dep_heads - 1, 0, L - 1)
    gathered = np.take_along_axis(bf_e, idx[:, :, None], axis=1)
    concat_embeds = np.where((dep_heads > 0)[:, :, None], gathered, 0.0).astype(np.float32)

    def lin(x, pp):
        return x @ np.asarray(pp['w'], np.float32).T + np.asarray(pp['b'], np.float32)

    dep_tag_space = lin(np.tanh(lin(bf_e, p['h2t_M']) + lin(concat_embeds, p['h2t_H'])),
                        p['mlp']).reshape(B * L, -1)
    pred = h3[np.arange(B), np.asarray(inputs['target_idx_in'])]
    hs = np.concatenate([h3, np.broadcast_to(pred[:, None, :], h3.shape)], axis=-1)
    role = np.concatenate([np.asarray(p['role_emb'], np.float32)[np.asarray(inputs['local_roles_voc'])],
                           np.asarray(p['frame_emb'], np.float32)[np.asarray(inputs['frames'])]],
                          axis=-1)
    mapped = np.maximum(lin(role, p['role_map']), 0.0)
    tag_space = np.einsum('blh,brh->blr', hs, mapped)
    sub = (np.asarray(inputs['local_roles_mask'], np.float32) - 1.0) * _BIG
    tag_space = tag_space + sub[:, None, :]
    tag2 = tag_space.reshape(B * L, -1).astype(np.float32)

    def logsoftmax(x):
        m = x.max(axis=1, keepdims=True)
        e = np.exp(x - m)
        return (x - m) - np.log(e.sum(axis=1, keepdims=True))

    SRLprobs = np.exp(logsoftmax(tag2))

    def ce_ignore0(logits, tgt):
        logp = logsoftmax(logits)
        nll = -logp[np.arange(logits.shape[0]), tgt]
        m = (tgt != 0).astype(np.float32)
        return (nll * m).sum() / max(m.sum(), 1.0)

    dep_labels = np.argmax(dep_tag_space, axis=1)
    gold = np.asarray(inputs['dep_tags']).reshape(-1)
    all_l = np.float32((gold != 0).sum())
    wrong_l = np.float32(((dep_labels != gold) & (gold != 0)).sum())
    SRLloss = np.float32(ce_ignore0(tag2, np.asarray(inputs['targets']).reshape(-1)))
    DEPloss = np.float32(ce_ignore0(dep_tag_space, gold))
    return SRLloss, DEPloss, np.float32(SRLloss + DEPloss), SRLprobs.astype(np.float32), wrong_l, all_l


def kernel(**inputs):
    params = inputs['params']
    nc = _build_program()
    wmap = _prep_lstm_weights(params)

    def np32(x):
        return np.ascontiguousarray(np.asarray(x, np.float32))

    tables = {
        "emb_word": np32(params['word_emb']),
        "emb_fixed": np32(params['word_fixed']),
        "emb_pos": np32(params['pos_emb']),
        "emb_lemma": np32(params['p_lemma_emb']),
    }
    in_maps = []
    for c in range(NC):
        sl = slice(c * BL, (c + 1) * BL)
        m = dict(tables)
        m.update({k: v for k, v in wmap.items()})
        m["sent_i"] = np.ascontiguousarray(np.asarray(inputs['sentence'][sl], np.int32))
        m["psent_i"] = np.ascontiguousarray(np.asarray(inputs['p_sentence'][sl], np.int32))
        m["pos_i"] = np.ascontiguousarray(np.asarray(inputs['pos_tags'][sl], np.int32))
        m["lemma_i"] = np.ascontiguousarray(np.asarray(inputs['sent_pred_lemmas_idx'][sl], np.int32))
        m["region"] = np32(inputs['region_marks'][sl])
        i4v = np.zeros((4, 34), np.float32)
        for j, col in enumerate([0, 1, 32, 33]):
            i4v[j, col] = 1.0
        m["i4m"] = i4v
        m["onesrow"] = np.ones((1, 128), np.float32)
        in_maps.append(m)

    import os
    trace = bool(os.environ.get("BASS_TRACE"))
    res = run_bass_kernel_spmd(nc, in_maps, core_ids=list(range(NC)), trace=trace)
    globals()["LAST_RESULTS"] = res
    bf_e = np.zeros((B, L, 2 * H), np.float32)
    h3 = np.zeros((B, L, 2 * H), np.float32)
    for c in range(NC):
        r = res.results[c]
        sl = slice(c * BL, (c + 1) * BL)
        bf_e[sl, :, 0:H] = r["hseq_bfe_f"] * 0.5   # H=2h -> true h
        bf_e[sl, :, H:2 * H] = r["hseq_bfe_b"] * 0.5
        h3[sl, :, 0:H] = r["hseq_h3_f"] * 0.5
        h3[sl, :, H:2 * H] = r["hseq_h3_b"] * 0.5
    return _host_post(bf_e, h3, inputs, params)


# revision 21
# speedup vs baseline: 1.4708x; 1.1546x over previous
"""Trainium2 Bass kernel for nn_BiLSTMTagger (self-contained).

Strategy: data-parallel over batch across 8 NeuronCores (2 sentences/core).
On device per core: embedding gathers (indirect DMA), 4 BiLSTM layers
(2 shared + 2 SRL). Per layer both directions run fused in one
block-diagonal float32r matmul per timestep; all gate nonlinearities are a
single tanh(0.5x) ACT op via sigmoid(x)=0.5*tanh(x/2)+0.5 with weights
pre-scaled on host (cell state kept as C=2c, H=2h; consumers' weights are
pre-halved to compensate exactly).
Post-LSTM (dep-head MLP branch, role scoring, softmax, CE losses) currently
on host in fp32 (exact), fed by the device-produced BiLSTM sequences.
"""
import sys

sys.path.insert(0, '/opt/trn_rl_repo')
import numpy as np
import concourse.bass as bass
import concourse.tile as tile
from concourse import mybir
from concourse.bass import ts
from concourse.bass_utils import run_bass_kernel_spmd
from concourse.masks import make_identity

F32 = mybir.dt.float32
F32R = mybir.dt.float32r
I32 = mybir.dt.int32

B, L, H = 16, 128, 300
NC = 8
BL = B // NC          # 2 sentences per core
GP = 1280             # padded gate columns: i 0:300 | f 300:600 | o 600:900 | g 900:1200 | pad
K1 = 1024             # layer-1 input features padded (918 -> 8*128)
K2 = 640              # layers 2-4 input features padded (601 -> 5*128)
KH = 640              # recurrence K: Hf 0:300 | Hb 300:600 | pad (5*128)
HB_OFF = 64           # H/Hb tile column offset so Hb transposes land at base-64
NSL = [(0, 512), (512, 512), (1024, 256)]  # gate column slices (psum-bank aligned)
ABLATE = {}  # timing-experiment knobs (exp.py); empty in production


def _split_all_waits(nc, maxw=1):
    """This walrus build allows 1 sync-wait per instruction; hoist extras onto
    same-engine NOPs spliced before the offending instruction."""
    cnt = [0]
    for f in nc.m.functions:
        for bb in f.blocks:
            insts = bb.instructions
            if not any(
                i.sync_info is not None and i.sync_info.on_wait and len(i.sync_info.on_wait) > maxw
                for i in insts
            ):
                continue
            new = []
            for inst in insts:
                si = inst.sync_info
                if si is not None and si.on_wait and len(si.on_wait) > maxw:
                    waits = list(si.on_wait)
                    keep = waits[-maxw:]
                    extra = waits[:-maxw]
                    for j in range(0, len(extra), maxw):
                        cnt[0] += 1
                        nop = mybir.InstNoOp(
                            name=f"I-wsplit-{cnt[0]}",
                            engine=inst.engine,
                            bass_nofuse=True,
                            sync_info=mybir.SyncInfo(on_wait=extra[j:j + maxw], on_update=[]),
                        )
                        nc.register_instruction(nop)
                        new.append(nop)
                    inst.sync_info = mybir.SyncInfo(on_wait=keep, on_update=list(si.on_update or []))
                new.append(inst)
            bb.instructions = new


def _layer_scan(nc, tc, ctx, lname, xp_f, xp_b, whh_rhs_f, whh_rhs_b, i2r, ident,
                hseq_f, hseq_b, sbuf, psum_g, psum_t, stage_pool):
    """One BiLSTM layer, 128 steps. Two independent chains (dir f and b), each
    batch-major [2, *]; they pipeline across PE/ACT/DVE.

    xp_d: SBUF [128, 2*GP] token-major input projections, batch j at cols j*GP.
    whh_rhs_d: SBUF [128, 3*GP] f32r recurrent weights (K = Hf 0:300, 3 chunks).
    hseq_d: DRAM [BL, L, H] outputs (true time order, H=2h scale).
    """
    chains = []
    for d, xp, whh_rhs, hseq in (("f", xp_f, whh_rhs_f, hseq_f),
                                 ("b", xp_b, whh_rhs_b, hseq_b)):
        lhsT = sbuf.tile([128, 3 * 2], F32R, name=f"lhsT{d}", tag=f"scan_lhsT{d}")
        nc.gpsimd.memset(lhsT[:].bitcast(F32), 0.0)
        ctile = sbuf.tile([2, H], F32, name=f"C{d}", tag=f"scan_C{d}")
        nc.gpsimd.memset(ctile[:], 0.0)
        htile = sbuf.tile([2, H], F32R, name=f"H{d}", tag=f"scan_H{d}")
        nc.gpsimd.memset(htile[:].bitcast(F32), 0.0)
        tall = sbuf.tile([2, 1200], F32, name=f"tall{d}", tag=f"scan_tall{d}")
        u1 = sbuf.tile([2, H], F32, name=f"u1{d}", tag=f"scan_u1{d}")
        sf = sbuf.tile([2, H], F32, name=f"sf{d}", tag=f"scan_sf{d}")
        mm = sbuf.tile([2, H], F32, name=f"mm{d}", tag=f"scan_m{d}")
        thc = sbuf.tile([2, H], F32, name=f"thc{d}", tag=f"scan_thc{d}")
        chains.append((d, xp, whh_rhs, hseq, lhsT, ctile, htile, tall, u1, sf, mm, thc))

    for t in range(L):
        # phase 1: stage DMAs + all matmuls for both chains (keeps PE dense:
        # chain b's MMs run while chain f's ACT/DVE work drains)
        gates_by_d = {}
        for (d, xp, whh_rhs, hseq, lhsT, ctile, htile, tall, u1, sf, mm, thc) in chains:
            tt = t if d == "f" else L - 1 - t
            stage = stage_pool.tile([2, GP], F32R, tag=f"stage{d}")
            if not ABLATE.get("no_xp"):
                for j in range(2):
                    nc.sync.dma_start(out=stage[j:j + 1, :],
                                      in_=xp[tt:tt + 1, j * GP:(j + 1) * GP].bitcast(F32R))
            gates = psum_g.tile([2, GP], F32, space="PSUM", tag=f"gates{d}")
            gates_by_d[d] = gates
            for kc in range(3):
                for (n0, nn) in NSL:
                    nc.tensor.matmul(
                        out=gates[:, n0:n0 + nn],
                        lhsT=lhsT[:, 2 * kc:2 * kc + 2],
                        rhs=whh_rhs[:, GP * kc + n0:GP * kc + n0 + nn],
                        start=(kc == 0), stop=False,
                    )
            if not ABLATE.get("no_xp"):
                for si_, (n0, nn) in enumerate(NSL):
                    nc.tensor.matmul(
                        out=gates[:, n0:n0 + nn],
                        lhsT=i2r[:2, :2],
                        rhs=stage[:, n0:n0 + nn],
                        start=False, stop=(si_ == len(NSL) - 1),
                    )
            else:
                for si_, (n0, nn) in enumerate(NSL):
                    nc.tensor.matmul(
                        out=gates[:, n0:n0 + nn], lhsT=lhsT[:, 0:2],
                        rhs=whh_rhs[:, n0:n0 + nn], start=False,
                        stop=(si_ == len(NSL) - 1))
        # phase 2: cell updates, sub-op interleaved so neither chain's small
        # ACT op blocks the other chain's big tanh in the in-order ACT queue
        live = [c for c in chains]
        for (d, xp, whh_rhs, hseq, lhsT, ctile, htile, tall, u1, sf, mm, thc) in live:
            gates = gates_by_d[d]
            if ABLATE.get("no_cell"):
                nc.scalar.activation(out=htile[:].bitcast(F32), in_=gates[:, 0:300],
                                     func=mybir.ActivationFunctionType.Tanh, scale=0.5)
            else:
                nc.scalar.activation(out=tall[:], in_=gates[:, 0:1200],
                                     func=mybir.ActivationFunctionType.Tanh, scale=0.5)
        if not ABLATE.get("no_cell"):
            for (d, xp, whh_rhs, hseq, lhsT, ctile, htile, tall, u1, sf, mm, thc) in live:
                nc.vector.scalar_tensor_tensor(out=u1[:], in0=tall[:, 0:300], scalar=1.0,
                                               in1=tall[:, 900:1200],
                                               op0=mybir.AluOpType.add, op1=mybir.AluOpType.mult)
                nc.vector.tensor_scalar(out=sf[:], in0=tall[:, 300:600], scalar1=0.5, scalar2=0.5,
                                        op0=mybir.AluOpType.mult, op1=mybir.AluOpType.add)
            for (d, xp, whh_rhs, hseq, lhsT, ctile, htile, tall, u1, sf, mm, thc) in live:
                nc.vector.tensor_tensor(out=mm[:], in0=sf[:], in1=ctile[:], op=mybir.AluOpType.mult)
                nc.vector.tensor_tensor(out=ctile[:], in0=mm[:], in1=u1[:], op=mybir.AluOpType.add)
            for (d, xp, whh_rhs, hseq, lhsT, ctile, htile, tall, u1, sf, mm, thc) in live:
                nc.scalar.activation(out=thc[:], in_=ctile[:],
                                     func=mybir.ActivationFunctionType.Tanh, scale=0.5)
            for (d, xp, whh_rhs, hseq, lhsT, ctile, htile, tall, u1, sf, mm, thc) in live:
                nc.vector.scalar_tensor_tensor(out=htile[:], in0=tall[:, 600:900],
                                               scalar=1.0, in1=thc[:],
                                               op0=mybir.AluOpType.add, op1=mybir.AluOpType.mult)
        # phase 3: H out + transposes for both chains
        for (d, xp, whh_rhs, hseq, lhsT, ctile, htile, tall, u1, sf, mm, thc) in chains:
            tt = t if d == "f" else L - 1 - t
            nc.gpsimd.dma_start(out=hseq[:, tt, :], in_=htile[:].bitcast(F32))
            if not ABLATE.get("no_transp"):
                for kc, (c0, cw) in enumerate([(0, 128), (128, 128), (256, 44)]):
                    tp = psum_t.tile([128, 2], F32, space="PSUM", tag="tp")
                    nc.tensor.transpose(out=tp[:cw, :], in_=htile[:, c0:c0 + cw].bitcast(F32),
                                        identity=ident[:2, :2])
                    nc.vector.tensor_copy(out=lhsT[0:cw, 2 * kc:2 * kc + 2], in_=tp[0:cw, :])


def _build_xT_from_hseq(nc, tc, hf_t, hb_t, xT, ident, psum_x, ones_t):
    """hf_t [128,304] (Hf at 0:300), hb_t [128,384] (Hb at cols 44:344) ->
    xT [128, 5*128] f32r: rows = features [Hf 0:300 | Hb 300:600 | ones 600 | pad]."""
    pieces = [
        (hf_t[:, 0:128], 0, 0, 128),
        (hf_t[:, 128:256], 1, 0, 128),
        (hf_t[:, 256:300], 2, 0, 44),
        (hb_t[:, 0:128], 2, 64, 128),      # Hb[0:64] -> feature rows 320:384
        (hb_t[:, 128:256], 3, 0, 128),     # Hb[64:192]
        (hb_t[:, 256:384], 4, 0, 128),     # Hb[192:300]+pad
    ]
    for (src, chunk, r0, r1) in pieces:
        tp = psum_x.tile([128, 128], F32, space="PSUM", tag="tp")
        nc.tensor.transpose(out=tp[:src.shape[1], :], in_=src, identity=ident[:])  # [128,*] full K
        nc.vector.tensor_copy(out=xT[r0:r1, ts(chunk, 128)], in_=tp[r0:r1, :])
    # ones row at feature 620 = chunk 4 row 108 (DMA: engines can't address base 108)
    nc.sync.dma_start(out=xT[108:109, ts(4, 128)], in_=ones_t[:, 0:128].bitcast(F32R))


def _proj(nc, xT, w_rhs, xp_out, col0, kcs, psum_p, sbuf_unused):
    """xp_out[:, col0:col0+GP] = (xT.T @ w_rhs) ; xT [128, kcs*128] f32r,
    w_rhs [128, kcs*GP] f32r, out token-major [128, GP] fp32."""
    pt = psum_p.tile([128, GP], F32, space="PSUM", tag="gatesf")
    for (n0, nn) in NSL:
        for kc in range(kcs):
            nc.tensor.matmul(
                out=pt[:, n0:n0 + nn],
                lhsT=xT[:, ts(kc, 128)],
                rhs=w_rhs[:, GP * kc + n0:GP * kc + n0 + nn],
                start=(kc == 0), stop=(kc == kcs - 1),
            )
    nc.vector.tensor_copy(out=xp_out[:, col0:col0 + GP], in_=pt[:])


_PROGRAM_CACHE = {}


def _build_program():
    if "nc" in _PROGRAM_CACHE:
        return _PROGRAM_CACHE["nc"]
    nc = bass.Bass()
    dp = nc.declare_dram_parameter
    # ---- inputs ----
    emb_word = dp("emb_word", [50000, 300], F32, isOutput=False)
    emb_fixed = dp("emb_fixed", [50000, 300], F32, isOutput=False)
    emb_pos = dp("emb_pos", [60, 16], F32, isOutput=False)
    emb_lemma = dp("emb_lemma", [5000, 300], F32, isOutput=False)
    sent_i = dp("sent_i", [BL, 128], I32, isOutput=False)
    psent_i = dp("psent_i", [BL, 128], I32, isOutput=False)
    pos_i = dp("pos_i", [BL, 128], I32, isOutput=False)
    lemma_i = dp("lemma_i", [BL, 128], I32, isOutput=False)
    region = dp("region", [BL, 128], F32, isOutput=False)
    i4m = dp("i4m", [4, 34], F32, isOutput=False)
    onesrow = dp("onesrow", [1, 128], F32, isOutput=False)
    wih = {}
    for l in range(1, 5):
        kc = 8 if l == 1 else 5
        for d in "fb":
            wih[(l, d)] = dp(f"wih{l}{d}", [128, kc * GP], F32, isOutput=False)
    whh = {(l, d): dp(f"whh{l}{d}", [128, 3 * GP], F32, isOutput=False)
           for l in range(1, 5) for d in "fb"}
    # ---- outputs: the four BiLSTM layer sequences we need downstream ----
    hseq = {}
    for l, n in [(2, "bfe"), (4, "h3")]:
        for d in "fb":
            hseq[(l, d)] = dp(f"hseq_{n}_{d}", [BL, L, H], F32, isOutput=True)
    for l in [1, 3]:
        for d in "fb":
            hseq[(l, d)] = nc.dram_tensor(f"hseq_l{l}_{d}", [BL, L, H], F32)

    with tile.TileContext(nc) as tc:
        import contextlib
        ctx = contextlib.ExitStack()
        with ctx:
            sbuf = ctx.enter_context(tc.tile_pool(name="sbuf", bufs=1))
            psum_g = ctx.enter_context(tc.tile_pool(name="psg", bufs=1, space="PSUM"))
            psum_t = ctx.enter_context(tc.tile_pool(name="pst", bufs=2, space="PSUM"))
            stage_pool = ctx.enter_context(tc.tile_pool(name="stage", bufs=4))
            wpool = ctx.enter_context(tc.tile_pool(name="wpool", bufs=1))

            ident = sbuf.tile([128, 128], F32, tag="ident")
            make_identity(nc, ident[:])
            ones_t = sbuf.tile([1, 128], F32, tag="ones_t")
            nc.sync.dma_start(out=ones_t[:], in_=onesrow[:])
            identr = sbuf.tile([128, 128], F32R, tag="identr")
            nc.vector.tensor_copy(out=identr[:], in_=ident[:])

            # ---- embeddings gather -> x [128, 1024] per b; transpose -> x1T ----
            x1T = [sbuf.tile([128, 8 * 128], F32R, name=f"x1T{b}", tag=f"x1T{b}") for b in range(BL)]
            for b in range(BL):
                xt = sbuf.tile([128, K1], F32, tag="xgath")
                nc.gpsimd.memset(xt[:], 0.0)
                for (idx_t, table, c0, w) in [
                    (sent_i, emb_word, 0, 300),
                    (psent_i, emb_fixed, 300, 300),
                    (pos_i, emb_pos, 600, 16),
                    (lemma_i, emb_lemma, 616, 300),
                ]:
                    it = sbuf.tile([128, 1], I32, tag="idx")
                    nc.sync.dma_start(out=it[:, 0:1], in_=idx_t[b, :, None])
                    nc.gpsimd.indirect_dma_start(
                        out=xt[:, c0:c0 + w], out_offset=None, in_=table[:],
                        in_offset=bass.IndirectOffsetOnAxis(ap=it[:, :1], axis=0),
                    )
                nc.sync.dma_start(out=xt[:, 916:917], in_=region[b, :, None])
                nc.gpsimd.memset(xt[:, 917:918], 1.0)
                for kc in range(8):
                    tp = psum_t.tile([128, 128], F32, space="PSUM", tag="tp")
                    nc.tensor.transpose(out=tp[:], in_=xt[:, ts(kc, 128)], identity=ident[:])
                    nc.vector.tensor_copy(out=x1T[b][:, ts(kc, 128)], in_=tp[:])

            # ---- per-layer: projections then scan ----
            hf_t = [sbuf.tile([128, 304], F32, name=f"hf{b}", tag=f"hf{b}") for b in range(BL)]
            hb_t = [sbuf.tile([128, 384], F32, name=f"hb{b}", tag=f"hb{b}") for b in range(BL)]
            x2T = [sbuf.tile([128, 5 * 128], F32R, name=f"x2T{b}", tag=f"x2T{b}") for b in range(BL)]
            for b in range(BL):
                nc.gpsimd.memset(hb_t[b][:], 0.0)
                nc.gpsimd.memset(x2T[b][:].bitcast(F32), 0.0)

            for l in range(1, 5):
                kc = 8 if l == 1 else 5
                xp_f = sbuf.tile([128, 2 * GP], F32, tag="xp_f")
                xp_b = sbuf.tile([128, 2 * GP], F32, tag="xp_b")
                wtile = wpool.tile([128, kc * GP], F32R, tag="wtile")
                for d, xp in (("f", xp_f), ("b", xp_b)):
                    nc.sync.dma_start(out=wtile[:], in_=wih[(l, d)][:].bitcast(F32R))
                    for b in range(BL):
                        if l == 1:
                            xT = x1T[b]
                        else:
                            xT = x2T[b]
                        _proj(nc, xT, wtile, xp, b * GP, kc, psum_g, sbuf)
                whh_rhs_f = wpool.tile([128, 3 * GP], F32R, tag="whh_rhs_f")
                nc.sync.dma_start(out=whh_rhs_f[:], in_=whh[(l, "f")][:].bitcast(F32R))
                whh_rhs_b = wpool.tile([128, 3 * GP], F32R, tag="whh_rhs_b")
                nc.sync.dma_start(out=whh_rhs_b[:], in_=whh[(l, "b")][:].bitcast(F32R))
                if not ABLATE.get("no_scan"):
                    _layer_scan(nc, tc, ctx, f"l{l}", xp_f, xp_b, whh_rhs_f, whh_rhs_b,
                                identr, ident,
                                hseq[(l, "f")], hseq[(l, "b")], sbuf, psum_g, psum_t, stage_pool)
                if l < 4:
                    for b in range(BL):
                        nc.sync.dma_start(out=hf_t[b][:, 0:300], in_=hseq[(l, "f")][b, :, :])
                        nc.sync.dma_start(out=hb_t[b][:, HB_OFF:HB_OFF + 300],  # Hb at cols 64:364
                                          in_=hseq[(l, "b")][b, :, :])
                        _build_xT_from_hseq(nc, tc, hf_t[b], hb_t[b], x2T[b], ident, psum_t, ones_t)

    _split_all_waits(nc)
    _PROGRAM_CACHE["nc"] = nc
    return nc


def _prep_lstm_weights(params):
    """Reorder gates i,f,g,o -> i,f,o,g; double g rows (tanh(0.5*2x)=tanh x);
    scale input weights of consumers of H=2h by 0.5; fold bias as ones-feature."""
    def reorder(w):
        i, f, g, o = np.split(w, 4, axis=0)
        return np.concatenate([i, f, o, 2.0 * g], axis=0)

    out = {}
    layers = list(params['lstm_share']) + list(params['lstm_srl'])
    for li, dirs in enumerate(layers, start=1):
        kc = 8 if li == 1 else 5
        for di, dname in enumerate("fb"):
            p = dirs[di]
            wih = reorder(np.asarray(p['wih'], np.float32))      # [1200, din]
            bb_ = reorder(np.asarray(p['b'], np.float32)[:, None])[:, 0]
            whh_ = reorder(np.asarray(p['whh'], np.float32))     # [1200, 300]
            din = wih.shape[1]
            if li > 1:
                wih = wih * 0.5           # input is H=2h
            whh_ = whh_ * 0.5             # recurrent input is H=2h
            wr = np.zeros((kc * 128, GP), np.float32)
            if li == 1:
                wr[:din, :1200] = wih.T
                wr[917, :1200] = bb_      # ones feature at col 917
            else:
                wr[0:300, :1200] = wih.T[0:300]      # Hf features
                wr[320:620, :1200] = wih.T[300:600]  # Hb features
                wr[620, :1200] = bb_                 # ones feature
            out[f"wih{li}{dname}"] = np.ascontiguousarray(
                wr.reshape(kc, 128, GP).transpose(1, 0, 2).reshape(128, kc * GP))
            whh_blk = np.zeros((3 * 128, GP), np.float32)
            whh_blk[0:300, :1200] = whh_.T
            out[f"whh{li}{dname}"] = np.ascontiguousarray(
                whh_blk.reshape(3, 128, GP).transpose(1, 0, 2).reshape(128, 3 * GP))
    return out


def _host_post(bf_e, h3, inputs, params):
    """Exact fp32 post-LSTM math on host. bf_e/h3: [B, L, 600] true scale."""
    p = params
    _BIG = 10.0 ** 6.0
    dep_heads = np.asarray(inputs['dep_heads'])
    idx = np.clip(dep_heads - 1, 0, L - 1)
    gathered = np.take_along_axis(bf_e, idx[:, :, None], axis=1)
    concat_embeds = np.where((dep_heads > 0)[:, :, None], gathered, 0.0).astype(np.float32)

    def lin(x, pp):
        return x @ np.asarray(pp['w'], np.float32).T + np.asarray(pp['b'], np.float32)

    dep_tag_space = lin(np.tanh(lin(bf_e, p['h2t_M']) + lin(concat_embeds, p['h2t_H'])),
                        p['mlp']).reshape(B * L, -1)
    pred = h3[np.arange(B), np.asarray(inputs['target_idx_in'])]
    hs = np.concatenate([h3, np.broadcast_to(pred[:, None, :], h3.shape)], axis=-1)
    role = np.concatenate([np.asarray(p['role_emb'], np.float32)[np.asarray(inputs['local_roles_voc'])],
                           np.asarray(p['frame_emb'], np.float32)[np.asarray(inputs['frames'])]],
                          axis=-1)
    mapped = np.maximum(lin(role, p['role_map']), 0.0)
    tag_space = np.einsum('blh,brh->blr', hs, mapped)
    sub = (np.asarray(inputs['local_roles_mask'], np.float32) - 1.0) * _BIG
    tag_space = tag_space + sub[:, None, :]
    tag2 = tag_space.reshape(B * L, -1).astype(np.float32)

    def logsoftmax(x):
        m = x.max(axis=1, keepdims=True)
        e = np.exp(x - m)
        return (x - m) - np.log(e.sum(axis=1, keepdims=True))

    SRLprobs = np.exp(logsoftmax(tag2))

    def ce_ignore0(logits, tgt):
        logp = logsoftmax(logits)
        nll = -logp[np.arange(logits.shape[0]), tgt]
        m = (tgt != 0).astype(np.float32)
        return (nll * m).sum() / max(m.sum(), 1.0)

    dep_labels = np.argmax(dep_tag_space, axis=1)
    gold = np.asarray(inputs['dep_tags']).reshape(-1)
    all_l = np.float32((gold != 0).sum())
    wrong_l = np.float32(((dep_labels != gold) & (gold != 0)).sum())
    SRLloss = np.float32(ce_ignore0(tag2, np.asarray(inputs['targets']).reshape(-1)))
    DEPloss = np.float32(ce_ignore0(dep_tag_space, gold))
    return SRLloss, DEPloss, np.float32(SRLloss + DEPloss), SRLprobs.astype(np.float32), wrong_l, all_l


def kernel(**inputs):
    params = inputs['params']
    nc = _build_program()
    wmap = _prep_lstm_weights(params)

    def np32(x):
        return np.ascontiguousarray(np.asarray(x, np.float32))

    tables = {
        "emb_word": np32(params['word_emb']),
        "emb_fixed": np32(params['word_fixed']),
        "emb_pos": np32(params['pos_emb']),
        "emb_lemma": np32(params['p_lemma_emb']),
    }
    in_maps = []
    for c in range(NC):
        sl = slice(c * BL, (c + 1) * BL)
        m = dict(tables)
        m.update({k: v for k, v in wmap.items()})
        m["sent_i"] = np.ascontiguousarray(np.asarray(inputs['sentence'][sl], np.int32))
        m["psent_i"] = np.ascontiguousarray(np.asarray(inputs['p_sentence'][sl], np.int32))
        m["pos_i"] = np.ascontiguousarray(np.asarray(inputs['pos_tags'][sl], np.int32))
        m["lemma_i"] = np.ascontiguousarray(np.asarray(inputs['sent_pred_lemmas_idx'][sl], np.int32))
        m["region"] = np32(inputs['region_marks'][sl])
        i4v = np.zeros((4, 34), np.float32)
        for j, col in enumerate([0, 1, 32, 33]):
            i4v[j, col] = 1.0
        m["i4m"] = i4v
        m["onesrow"] = np.ones((1, 128), np.float32)
        in_maps.append(m)

    import os
    trace = bool(os.environ.get("BASS_TRACE"))
    res = run_bass_kernel_spmd(nc, in_maps, core_ids=list(range(NC)), trace=trace)
    globals()["LAST_RESULTS"] = res
    bf_e = np.zeros((B, L, 2 * H), np.float32)
    h3 = np.zeros((B, L, 2 * H), np.float32)
    for c in range(NC):
        r = res.results[c]
        sl = slice(c * BL, (c + 1) * BL)
        bf_e[sl, :, 0:H] = r["hseq_bfe_f"] * 0.5   # H=2h -> true h
        bf_e[sl, :, H:2 * H] = r["hseq_bfe_b"] * 0.5
        h3[sl, :, 0:H] = r["hseq_h3_f"] * 0.5
        h3[sl, :, H:2 * H] = r["hseq_h3_b"] * 0.5
    return _host_post(bf_e, h3, inputs, params)


# revision 22
# speedup vs baseline: 1.4733x; 1.0016x over previous
"""Trainium2 Bass kernel for nn_BiLSTMTagger (self-contained).

Strategy: data-parallel over batch across 8 NeuronCores (2 sentences/core).
On device per core: embedding gathers (indirect DMA), 4 BiLSTM layers
(2 shared + 2 SRL). Per layer both directions run fused in one
block-diagonal float32r matmul per timestep; all gate nonlinearities are a
single tanh(0.5x) ACT op via sigmoid(x)=0.5*tanh(x/2)+0.5 with weights
pre-scaled on host (cell state kept as C=2c, H=2h; consumers' weights are
pre-halved to compensate exactly).
Post-LSTM (dep-head MLP branch, role scoring, softmax, CE losses) currently
on host in fp32 (exact), fed by the device-produced BiLSTM sequences.
"""
import sys

sys.path.insert(0, '/opt/trn_rl_repo')
import numpy as np
import concourse.bass as bass
import concourse.tile as tile
from concourse import mybir
from concourse.bass import ts
from concourse.bass_utils import run_bass_kernel_spmd
from concourse.masks import make_identity

F32 = mybir.dt.float32
F32R = mybir.dt.float32r
I32 = mybir.dt.int32

B, L, H = 16, 128, 300
NC = 8
BL = B // NC          # 2 sentences per core
GP = 1280             # padded gate columns: i 0:300 | f 300:600 | o 600:900 | g 900:1200 | pad
K1 = 1024             # layer-1 input features padded (918 -> 8*128)
K2 = 640              # layers 2-4 input features padded (601 -> 5*128)
KH = 640              # recurrence K: Hf 0:300 | Hb 300:600 | pad (5*128)
HB_OFF = 64           # H/Hb tile column offset so Hb transposes land at base-64
NSL = [(0, 512), (512, 512), (1024, 256)]  # gate column slices (psum-bank aligned)
ABLATE = {}  # timing-experiment knobs (exp.py); empty in production


def _split_all_waits(nc, maxw=1):
    """This walrus build allows 1 sync-wait per instruction; hoist extras onto
    same-engine NOPs spliced before the offending instruction."""
    cnt = [0]
    for f in nc.m.functions:
        for bb in f.blocks:
            insts = bb.instructions
            if not any(
                i.sync_info is not None and i.sync_info.on_wait and len(i.sync_info.on_wait) > maxw
                for i in insts
            ):
                continue
            new = []
            for inst in insts:
                si = inst.sync_info
                if si is not None and si.on_wait and len(si.on_wait) > maxw:
                    waits = list(si.on_wait)
                    keep = waits[-maxw:]
                    extra = waits[:-maxw]
                    for j in range(0, len(extra), maxw):
                        cnt[0] += 1
                        nop = mybir.InstNoOp(
                            name=f"I-wsplit-{cnt[0]}",
                            engine=inst.engine,
                            bass_nofuse=True,
                            sync_info=mybir.SyncInfo(on_wait=extra[j:j + maxw], on_update=[]),
                        )
                        nc.register_instruction(nop)
                        new.append(nop)
                    inst.sync_info = mybir.SyncInfo(on_wait=keep, on_update=list(si.on_update or []))
                new.append(inst)
            bb.instructions = new


def _layer_scan(nc, tc, ctx, lname, xp_f, xp_b, whh_rhs_f, whh_rhs_b, i2r, ident,
                hseq_f, hseq_b, sbuf, psum_g, psum_t, stage_pool):
    """One BiLSTM layer, 128 steps. Two independent chains (dir f and b), each
    batch-major [2, *]; they pipeline across PE/ACT/DVE.

    xp_d: SBUF [128, 2*GP] token-major input projections, batch j at cols j*GP.
    whh_rhs_d: SBUF [128, 3*GP] f32r recurrent weights (K = Hf 0:300, 3 chunks).
    hseq_d: DRAM [BL, L, H] outputs (true time order, H=2h scale).
    """
    chains = []
    for d, xp, whh_rhs, hseq in (("f", xp_f, whh_rhs_f, hseq_f),
                                 ("b", xp_b, whh_rhs_b, hseq_b)):
        lhsT = sbuf.tile([128, 3 * 2], F32R, name=f"lhsT{d}", tag=f"scan_lhsT{d}")
        nc.gpsimd.memset(lhsT[:].bitcast(F32), 0.0)
        ctile = sbuf.tile([2, H], F32, name=f"C{d}", tag=f"scan_C{d}")
        nc.gpsimd.memset(ctile[:], 0.0)
        htile = None  # per-step pool tile (double-buffered, avoids WAR on hseq store)
        tall = sbuf.tile([2, 1200], F32, name=f"tall{d}", tag=f"scan_tall{d}")
        u1 = sbuf.tile([2, H], F32, name=f"u1{d}", tag=f"scan_u1{d}")
        sf = sbuf.tile([2, H], F32, name=f"sf{d}", tag=f"scan_sf{d}")
        mm = sbuf.tile([2, H], F32, name=f"mm{d}", tag=f"scan_m{d}")
        thc = sbuf.tile([2, H], F32, name=f"thc{d}", tag=f"scan_thc{d}")
        chains.append((d, xp, whh_rhs, hseq, lhsT, ctile, htile, tall, u1, sf, mm, thc))

    for t in range(L):
        # phase 1: stage DMAs + all matmuls for both chains (keeps PE dense:
        # chain b's MMs run while chain f's ACT/DVE work drains)
        gates_by_d = {}
        for (d, xp, whh_rhs, hseq, lhsT, ctile, htile, tall, u1, sf, mm, thc) in chains:
            tt = t if d == "f" else L - 1 - t
            stage = stage_pool.tile([2, GP], F32R, tag=f"stage{d}")
            if not ABLATE.get("no_xp"):
                for j in range(2):
                    nc.sync.dma_start(out=stage[j:j + 1, :],
                                      in_=xp[tt:tt + 1, j * GP:(j + 1) * GP].bitcast(F32R))
            gates = psum_g.tile([2, GP], F32, space="PSUM", tag=f"gates{d}")
            gates_by_d[d] = gates
            for kc in range(3):
                for (n0, nn) in NSL:
                    nc.tensor.matmul(
                        out=gates[:, n0:n0 + nn],
                        lhsT=lhsT[:, 2 * kc:2 * kc + 2],
                        rhs=whh_rhs[:, GP * kc + n0:GP * kc + n0 + nn],
                        start=(kc == 0), stop=False,
                    )
            if not ABLATE.get("no_xp"):
                for si_, (n0, nn) in enumerate(NSL):
                    nc.tensor.matmul(
                        out=gates[:, n0:n0 + nn],
                        lhsT=i2r[:2, :2],
                        rhs=stage[:, n0:n0 + nn],
                        start=False, stop=(si_ == len(NSL) - 1),
                    )
            else:
                for si_, (n0, nn) in enumerate(NSL):
                    nc.tensor.matmul(
                        out=gates[:, n0:n0 + nn], lhsT=lhsT[:, 0:2],
                        rhs=whh_rhs[:, n0:n0 + nn], start=False,
                        stop=(si_ == len(NSL) - 1))
        # phase 2: cell updates, sub-op interleaved so neither chain's small
        # ACT op blocks the other chain's big tanh in the in-order ACT queue
        live = [c for c in chains]
        h_by_d = {}
        for (d, xp, whh_rhs, hseq, lhsT, ctile, htile, tall, u1, sf, mm, thc) in live:
            gates = gates_by_d[d]
            htile = h_by_d[d] = stage_pool.tile([2, H], F32R, name=f"H{d}", tag=f"scan_H{d}")
            if ABLATE.get("no_cell"):
                nc.scalar.activation(out=htile[:].bitcast(F32), in_=gates[:, 0:300],
                                     func=mybir.ActivationFunctionType.Tanh, scale=0.5)
            else:
                nc.scalar.activation(out=tall[:], in_=gates[:, 0:1200],
                                     func=mybir.ActivationFunctionType.Tanh, scale=0.5)
        if not ABLATE.get("no_cell"):
            for (d, xp, whh_rhs, hseq, lhsT, ctile, htile, tall, u1, sf, mm, thc) in live:
                nc.vector.scalar_tensor_tensor(out=u1[:], in0=tall[:, 0:300], scalar=1.0,
                                               in1=tall[:, 900:1200],
                                               op0=mybir.AluOpType.add, op1=mybir.AluOpType.mult)
                nc.vector.tensor_scalar(out=sf[:], in0=tall[:, 300:600], scalar1=0.5, scalar2=0.5,
                                        op0=mybir.AluOpType.mult, op1=mybir.AluOpType.add)
            for (d, xp, whh_rhs, hseq, lhsT, ctile, htile, tall, u1, sf, mm, thc) in live:
                nc.vector.tensor_tensor(out=mm[:], in0=sf[:], in1=ctile[:], op=mybir.AluOpType.mult)
                nc.vector.tensor_tensor(out=ctile[:], in0=mm[:], in1=u1[:], op=mybir.AluOpType.add)
            for (d, xp, whh_rhs, hseq, lhsT, ctile, htile, tall, u1, sf, mm, thc) in live:
                nc.scalar.activation(out=thc[:], in_=ctile[:],
                                     func=mybir.ActivationFunctionType.Tanh, scale=0.5)
            for (d, xp, whh_rhs, hseq, lhsT, ctile, htile, tall, u1, sf, mm, thc) in live:
                nc.vector.scalar_tensor_tensor(out=h_by_d[d][:], in0=tall[:, 600:900],
                                               scalar=1.0, in1=thc[:],
                                               op0=mybir.AluOpType.add, op1=mybir.AluOpType.mult)
        # phase 3: H out + transposes for both chains
        for (d, xp, whh_rhs, hseq, lhsT, ctile, htile, tall, u1, sf, mm, thc) in chains:
            tt = t if d == "f" else L - 1 - t
            htile = h_by_d[d]
            nc.gpsimd.dma_start(out=hseq[:, tt, :], in_=htile[:].bitcast(F32))
            if not ABLATE.get("no_transp"):
                for kc, (c0, cw) in enumerate([(0, 128), (128, 128), (256, 44)]):
                    tp = psum_t.tile([128, 2], F32, space="PSUM", tag="tp")
                    nc.tensor.transpose(out=tp[:cw, :], in_=htile[:, c0:c0 + cw].bitcast(F32),
                                        identity=ident[:2, :2])
                    nc.vector.tensor_copy(out=lhsT[0:cw, 2 * kc:2 * kc + 2], in_=tp[0:cw, :])


def _build_xT_from_hseq(nc, tc, hf_t, hb_t, xT, ident, psum_x, ones_t):
    """hf_t [128,304] (Hf at 0:300), hb_t [128,384] (Hb at cols 44:344) ->
    xT [128, 5*128] f32r: rows = features [Hf 0:300 | Hb 300:600 | ones 600 | pad]."""
    pieces = [
        (hf_t[:, 0:128], 0, 0, 128),
        (hf_t[:, 128:256], 1, 0, 128),
        (hf_t[:, 256:300], 2, 0, 44),
        (hb_t[:, 0:128], 2, 64, 128),      # Hb[0:64] -> feature rows 320:384
        (hb_t[:, 128:256], 3, 0, 128),     # Hb[64:192]
        (hb_t[:, 256:384], 4, 0, 128),     # Hb[192:300]+pad
    ]
    for (src, chunk, r0, r1) in pieces:
        tp = psum_x.tile([128, 128], F32, space="PSUM", tag="tp")
        nc.tensor.transpose(out=tp[:src.shape[1], :], in_=src, identity=ident[:])  # [128,*] full K
        nc.vector.tensor_copy(out=xT[r0:r1, ts(chunk, 128)], in_=tp[r0:r1, :])
    # ones row at feature 620 = chunk 4 row 108 (DMA: engines can't address base 108)
    nc.sync.dma_start(out=xT[108:109, ts(4, 128)], in_=ones_t[:, 0:128].bitcast(F32R))


def _proj(nc, xT, w_rhs, xp_out, col0, kcs, psum_p, sbuf_unused):
    """xp_out[:, col0:col0+GP] = (xT.T @ w_rhs) ; xT [128, kcs*128] f32r,
    w_rhs [128, kcs*GP] f32r, out token-major [128, GP] fp32."""
    pt = psum_p.tile([128, GP], F32, space="PSUM", tag="gatesf")
    for (n0, nn) in NSL:
        for kc in range(kcs):
            nc.tensor.matmul(
                out=pt[:, n0:n0 + nn],
                lhsT=xT[:, ts(kc, 128)],
                rhs=w_rhs[:, GP * kc + n0:GP * kc + n0 + nn],
                start=(kc == 0), stop=(kc == kcs - 1),
            )
    nc.vector.tensor_copy(out=xp_out[:, col0:col0 + GP], in_=pt[:])


_PROGRAM_CACHE = {}


def _build_program():
    if "nc" in _PROGRAM_CACHE:
        return _PROGRAM_CACHE["nc"]
    nc = bass.Bass()
    dp = nc.declare_dram_parameter
    # ---- inputs ----
    emb_word = dp("emb_word", [50000, 300], F32, isOutput=False)
    emb_fixed = dp("emb_fixed", [50000, 300], F32, isOutput=False)
    emb_pos = dp("emb_pos", [60, 16], F32, isOutput=False)
    emb_lemma = dp("emb_lemma", [5000, 300], F32, isOutput=False)
    sent_i = dp("sent_i", [BL, 128], I32, isOutput=False)
    psent_i = dp("psent_i", [BL, 128], I32, isOutput=False)
    pos_i = dp("pos_i", [BL, 128], I32, isOutput=False)
    lemma_i = dp("lemma_i", [BL, 128], I32, isOutput=False)
    region = dp("region", [BL, 128], F32, isOutput=False)
    i4m = dp("i4m", [4, 34], F32, isOutput=False)
    onesrow = dp("onesrow", [1, 128], F32, isOutput=False)
    wih = {}
    for l in range(1, 5):
        kc = 8 if l == 1 else 5
        for d in "fb":
            wih[(l, d)] = dp(f"wih{l}{d}", [128, kc * GP], F32, isOutput=False)
    whh = {(l, d): dp(f"whh{l}{d}", [128, 3 * GP], F32, isOutput=False)
           for l in range(1, 5) for d in "fb"}
    # ---- outputs: the four BiLSTM layer sequences we need downstream ----
    hseq = {}
    for l, n in [(2, "bfe"), (4, "h3")]:
        for d in "fb":
            hseq[(l, d)] = dp(f"hseq_{n}_{d}", [BL, L, H], F32, isOutput=True)
    for l in [1, 3]:
        for d in "fb":
            hseq[(l, d)] = nc.dram_tensor(f"hseq_l{l}_{d}", [BL, L, H], F32)

    with tile.TileContext(nc) as tc:
        import contextlib
        ctx = contextlib.ExitStack()
        with ctx:
            sbuf = ctx.enter_context(tc.tile_pool(name="sbuf", bufs=1))
            psum_g = ctx.enter_context(tc.tile_pool(name="psg", bufs=1, space="PSUM"))
            psum_t = ctx.enter_context(tc.tile_pool(name="pst", bufs=2, space="PSUM"))
            stage_pool = ctx.enter_context(tc.tile_pool(name="stage", bufs=4))
            wpool = ctx.enter_context(tc.tile_pool(name="wpool", bufs=1))

            ident = sbuf.tile([128, 128], F32, tag="ident")
            make_identity(nc, ident[:])
            ones_t = sbuf.tile([1, 128], F32, tag="ones_t")
            nc.sync.dma_start(out=ones_t[:], in_=onesrow[:])
            identr = sbuf.tile([128, 128], F32R, tag="identr")
            nc.vector.tensor_copy(out=identr[:], in_=ident[:])

            # ---- embeddings gather -> x [128, 1024] per b; transpose -> x1T ----
            x1T = [sbuf.tile([128, 8 * 128], F32R, name=f"x1T{b}", tag=f"x1T{b}") for b in range(BL)]
            for b in range(BL):
                xt = sbuf.tile([128, K1], F32, tag="xgath")
                nc.gpsimd.memset(xt[:], 0.0)
                for (idx_t, table, c0, w) in [
                    (sent_i, emb_word, 0, 300),
                    (psent_i, emb_fixed, 300, 300),
                    (pos_i, emb_pos, 600, 16),
                    (lemma_i, emb_lemma, 616, 300),
                ]:
                    it = sbuf.tile([128, 1], I32, tag="idx")
                    nc.sync.dma_start(out=it[:, 0:1], in_=idx_t[b, :, None])
                    nc.gpsimd.indirect_dma_start(
                        out=xt[:, c0:c0 + w], out_offset=None, in_=table[:],
                        in_offset=bass.IndirectOffsetOnAxis(ap=it[:, :1], axis=0),
                    )
                nc.sync.dma_start(out=xt[:, 916:917], in_=region[b, :, None])
                nc.gpsimd.memset(xt[:, 917:918], 1.0)
                for kc in range(8):
                    tp = psum_t.tile([128, 128], F32, space="PSUM", tag="tp")
                    nc.tensor.transpose(out=tp[:], in_=xt[:, ts(kc, 128)], identity=ident[:])
                    nc.vector.tensor_copy(out=x1T[b][:, ts(kc, 128)], in_=tp[:])

            # ---- per-layer: projections then scan ----
            hf_t = [sbuf.tile([128, 304], F32, name=f"hf{b}", tag=f"hf{b}") for b in range(BL)]
            hb_t = [sbuf.tile([128, 384], F32, name=f"hb{b}", tag=f"hb{b}") for b in range(BL)]
            x2T = [sbuf.tile([128, 5 * 128], F32R, name=f"x2T{b}", tag=f"x2T{b}") for b in range(BL)]
            for b in range(BL):
                nc.gpsimd.memset(hb_t[b][:], 0.0)
                nc.gpsimd.memset(x2T[b][:].bitcast(F32), 0.0)

            for l in range(1, 5):
                kc = 8 if l == 1 else 5
                xp_f = sbuf.tile([128, 2 * GP], F32, tag="xp_f")
                xp_b = sbuf.tile([128, 2 * GP], F32, tag="xp_b")
                wtile = wpool.tile([128, kc * GP], F32R, tag="wtile")
                for d, xp in (("f", xp_f), ("b", xp_b)):
                    nc.sync.dma_start(out=wtile[:], in_=wih[(l, d)][:].bitcast(F32R))
                    for b in range(BL):
                        if l == 1:
                            xT = x1T[b]
                        else:
                            xT = x2T[b]
                        _proj(nc, xT, wtile, xp, b * GP, kc, psum_g, sbuf)
                whh_rhs_f = wpool.tile([128, 3 * GP], F32R, tag="whh_rhs_f")
                nc.sync.dma_start(out=whh_rhs_f[:], in_=whh[(l, "f")][:].bitcast(F32R))
                whh_rhs_b = wpool.tile([128, 3 * GP], F32R, tag="whh_rhs_b")
                nc.sync.dma_start(out=whh_rhs_b[:], in_=whh[(l, "b")][:].bitcast(F32R))
                if not ABLATE.get("no_scan"):
                    _layer_scan(nc, tc, ctx, f"l{l}", xp_f, xp_b, whh_rhs_f, whh_rhs_b,
                                identr, ident,
                                hseq[(l, "f")], hseq[(l, "b")], sbuf, psum_g, psum_t, stage_pool)
                if l < 4:
                    for b in range(BL):
                        nc.sync.dma_start(out=hf_t[b][:, 0:300], in_=hseq[(l, "f")][b, :, :])
                        nc.sync.dma_start(out=hb_t[b][:, HB_OFF:HB_OFF + 300],  # Hb at cols 64:364
                                          in_=hseq[(l, "b")][b, :, :])
                        _build_xT_from_hseq(nc, tc, hf_t[b], hb_t[b], x2T[b], ident, psum_t, ones_t)

    _split_all_waits(nc)
    _PROGRAM_CACHE["nc"] = nc
    return nc


def _prep_lstm_weights(params):
    """Reorder gates i,f,g,o -> i,f,o,g; double g rows (tanh(0.5*2x)=tanh x);
    scale input weights of consumers of H=2h by 0.5; fold bias as ones-feature."""
    def reorder(w):
        i, f, g, o = np.split(w, 4, axis=0)
        return np.concatenate([i, f, o, 2.0 * g], axis=0)

    out = {}
    layers = list(params['lstm_share']) + list(params['lstm_srl'])
    for li, dirs in enumerate(layers, start=1):
        kc = 8 if li == 1 else 5
        for di, dname in enumerate("fb"):
            p = dirs[di]
            wih = reorder(np.asarray(p['wih'], np.float32))      # [1200, din]
            bb_ = reorder(np.asarray(p['b'], np.float32)[:, None])[:, 0]
            whh_ = reorder(np.asarray(p['whh'], np.float32))     # [1200, 300]
            din = wih.shape[1]
            if li > 1:
                wih = wih * 0.5           # input is H=2h
            whh_ = whh_ * 0.5             # recurrent input is H=2h
            wr = np.zeros((kc * 128, GP), np.float32)
            if li == 1:
                wr[:din, :1200] = wih.T
                wr[917, :1200] = bb_      # ones feature at col 917
            else:
                wr[0:300, :1200] = wih.T[0:300]      # Hf features
                wr[320:620, :1200] = wih.T[300:600]  # Hb features
                wr[620, :1200] = bb_                 # ones feature
            out[f"wih{li}{dname}"] = np.ascontiguousarray(
                wr.reshape(kc, 128, GP).transpose(1, 0, 2).reshape(128, kc * GP))
            whh_blk = np.zeros((3 * 128, GP), np.float32)
            whh_blk[0:300, :1200] = whh_.T
            out[f"whh{li}{dname}"] = np.ascontiguousarray(
                whh_blk.reshape(3, 128, GP).transpose(1, 0, 2).reshape(128, 3 * GP))
    return out


def _host_post(bf_e, h3, inputs, params):
    """Exact fp32 post-LSTM math on host. bf_e/h3: [B, L, 600] true scale."""
    p = params
    _BIG = 10.0 ** 6.0
    dep_heads = np.asarray(inputs['dep_heads'])
    idx = np.clip(dep_heads - 1, 0, L - 1)
    gathered = np.take_along_axis(bf_e, idx[:, :, None], axis=1)
    concat_embeds = np.where((dep_heads > 0)[:, :, None], gathered, 0.0).astype(np.float32)

    def lin(x, pp):
        return x @ np.asarray(pp['w'], np.float32).T + np.asarray(pp['b'], np.float32)

    dep_tag_space = lin(np.tanh(lin(bf_e, p['h2t_M']) + lin(concat_embeds, p['h2t_H'])),
                        p['mlp']).reshape(B * L, -1)
    pred = h3[np.arange(B), np.asarray(inputs['target_idx_in'])]
    hs = np.concatenate([h3, np.broadcast_to(pred[:, None, :], h3.shape)], axis=-1)
    role = np.concatenate([np.asarray(p['role_emb'], np.float32)[np.asarray(inputs['local_roles_voc'])],
                           np.asarray(p['frame_emb'], np.float32)[np.asarray(inputs['frames'])]],
                          axis=-1)
    mapped = np.maximum(lin(role, p['role_map']), 0.0)
    tag_space = np.einsum('blh,brh->blr', hs, mapped)
    sub = (np.asarray(inputs['local_roles_mask'], np.float32) - 1.0) * _BIG
    tag_space = tag_space + sub[:, None, :]
    tag2 = tag_space.reshape(B * L, -1).astype(np.float32)

    def logsoftmax(x):
        m = x.max(axis=1, keepdims=True)
        e = np.exp(x - m)
        return (x - m) - np.log(e.sum(axis=1, keepdims=True))

    SRLprobs = np.exp(logsoftmax(tag2))

    def ce_ignore0(logits, tgt):
        logp = logsoftmax(logits)
        nll = -logp[np.arange(logits.shape[0]), tgt]
        m = (tgt != 0).astype(np.float32)
        return (nll * m).sum() / max(m.sum(), 1.0)

    dep_labels = np.argmax(dep_tag_space, axis=1)
    gold = np.asarray(inputs['dep_tags']).reshape(-1)
    all_l = np.float32((gold != 0).sum())
    wrong_l = np.float32(((dep_labels != gold) & (gold != 0)).sum())
    SRLloss = np.float32(ce_ignore0(tag2, np.asarray(inputs['targets']).reshape(-1)))
    DEPloss = np.float32(ce_ignore0(dep_tag_space, gold))
    return SRLloss, DEPloss, np.float32(SRLloss + DEPloss), SRLprobs.astype(np.float32), wrong_l, all_l


def kernel(**inputs):
    params = inputs['params']
    nc = _build_program()
    wmap = _prep_lstm_weights(params)

    def np32(x):
        return np.ascontiguousarray(np.asarray(x, np.float32))

    tables = {
        "emb_word": np32(params['word_emb']),
        "emb_fixed": np32(params['word_fixed']),
        "emb_pos": np32(params['pos_emb']),
        "emb_lemma": np32(params['p_lemma_emb']),
    }
    in_maps = []
    for c in range(NC):
        sl = slice(c * BL, (c + 1) * BL)
        m = dict(tables)
        m.update({k: v for k, v in wmap.items()})
        m["sent_i"] = np.ascontiguousarray(np.asarray(inputs['sentence'][sl], np.int32))
        m["psent_i"] = np.ascontiguousarray(np.asarray(inputs['p_sentence'][sl], np.int32))
        m["pos_i"] = np.ascontiguousarray(np.asarray(inputs['pos_tags'][sl], np.int32))
        m["lemma_i"] = np.ascontiguousarray(np.asarray(inputs['sent_pred_lemmas_idx'][sl], np.int32))
        m["region"] = np32(inputs['region_marks'][sl])
        i4v = np.zeros((4, 34), np.float32)
        for j, col in enumerate([0, 1, 32, 33]):
            i4v[j, col] = 1.0
        m["i4m"] = i4v
        m["onesrow"] = np.ones((1, 128), np.float32)
        in_maps.append(m)

    import os
    trace = bool(os.environ.get("BASS_TRACE"))
    res = run_bass_kernel_spmd(nc, in_maps, core_ids=list(range(NC)), trace=trace)
    globals()["LAST_RESULTS"] = res
    bf_e = np.zeros((B, L, 2 * H), np.float32)
    h3 = np.zeros((B, L, 2 * H), np.float32)
    for c in range(NC):
        r = res.results[c]
        sl = slice(c * BL, (c + 1) * BL)
        bf_e[sl, :, 0:H] = r["hseq_bfe_f"] * 0.5   # H=2h -> true h
        bf_e[sl, :, H:2 * H] = r["hseq_bfe_b"] * 0.5
        h3[sl, :, 0:H] = r["hseq_h3_f"] * 0.5
        h3[sl, :, H:2 * H] = r["hseq_h3_b"] * 0.5
    return _host_post(bf_e, h3, inputs, params)


# revision 25
# speedup vs baseline: 1.5054x; 1.0218x over previous
"""Trainium2 Bass kernel for nn_BiLSTMTagger (self-contained).

Strategy: data-parallel over batch across 8 NeuronCores (2 sentences/core).
On device per core: embedding gathers (indirect DMA), 4 BiLSTM layers
(2 shared + 2 SRL). Per layer both directions run fused in one
block-diagonal float32r matmul per timestep; all gate nonlinearities are a
single tanh(0.5x) ACT op via sigmoid(x)=0.5*tanh(x/2)+0.5 with weights
pre-scaled on host (cell state kept as C=2c, H=2h; consumers' weights are
pre-halved to compensate exactly).
Post-LSTM (dep-head MLP branch, role scoring, softmax, CE losses) currently
on host in fp32 (exact), fed by the device-produced BiLSTM sequences.
"""
import sys

sys.path.insert(0, '/opt/trn_rl_repo')
import numpy as np
import concourse.bass as bass
import concourse.tile as tile
from concourse import mybir
from concourse.bass import ts
from concourse.bass_utils import run_bass_kernel_spmd
from concourse.masks import make_identity

F32 = mybir.dt.float32
F32R = mybir.dt.float32r
I32 = mybir.dt.int32

B, L, H = 16, 128, 300
NC = 8
BL = B // NC          # 2 sentences per core
GP = 1280             # padded gate columns: i 0:300 | f 300:600 | o 600:900 | g 900:1200 | pad
K1 = 1024             # layer-1 input features padded (918 -> 8*128)
K2 = 640              # layers 2-4 input features padded (601 -> 5*128)
KH = 640              # recurrence K: Hf 0:300 | Hb 300:600 | pad (5*128)
HB_OFF = 64           # H/Hb tile column offset so Hb transposes land at base-64
NSL = [(0, 512), (512, 512), (1024, 256)]  # gate column slices (psum-bank aligned)
ABLATE = {}  # timing-experiment knobs (exp.py); empty in production


def _split_all_waits(nc, maxw=1):
    """This walrus build allows 1 sync-wait per instruction; hoist extras onto
    same-engine NOPs spliced before the offending instruction."""
    cnt = [0]
    for f in nc.m.functions:
        for bb in f.blocks:
            insts = bb.instructions
            if not any(
                i.sync_info is not None and i.sync_info.on_wait and len(i.sync_info.on_wait) > maxw
                for i in insts
            ):
                continue
            new = []
            for inst in insts:
                si = inst.sync_info
                if si is not None and si.on_wait and len(si.on_wait) > maxw:
                    waits = list(si.on_wait)
                    keep = waits[-maxw:]
                    extra = waits[:-maxw]
                    for j in range(0, len(extra), maxw):
                        cnt[0] += 1
                        nop = mybir.InstNoOp(
                            name=f"I-wsplit-{cnt[0]}",
                            engine=inst.engine,
                            bass_nofuse=True,
                            sync_info=mybir.SyncInfo(on_wait=extra[j:j + maxw], on_update=[]),
                        )
                        nc.register_instruction(nop)
                        new.append(nop)
                    inst.sync_info = mybir.SyncInfo(on_wait=keep, on_update=list(si.on_update or []))
                new.append(inst)
            bb.instructions = new


def _layer_scan(nc, tc, ctx, lname, xp_f, xp_b, whh_rhs_f, whh_rhs_b, i2r, ident,
                hseq_f, hseq_b, sbuf, psum_g, psum_t, stage_pool):
    """One BiLSTM layer, 128 steps. Two independent chains (dir f and b), each
    batch-major [2, *]; they pipeline across PE/ACT/DVE.

    xp_d: SBUF [128, 2*GP] token-major input projections, batch j at cols j*GP.
    whh_rhs_d: SBUF [128, 3*GP] f32r recurrent weights (K = Hf 0:300, 3 chunks).
    hseq_d: DRAM [BL, L, H] outputs (true time order, H=2h scale).
    """
    chains = []
    for d, xp, whh_rhs, hseq in (("f", xp_f, whh_rhs_f, hseq_f),
                                 ("b", xp_b, whh_rhs_b, hseq_b)):
        lhsT = sbuf.tile([128, 3 * 2], F32R, name=f"lhsT{d}", tag=f"scan_lhsT{d}")
        nc.gpsimd.memset(lhsT[:].bitcast(F32), 0.0)
        ctile = sbuf.tile([2, H], F32, name=f"C{d}", tag=f"scan_C{d}")
        nc.gpsimd.memset(ctile[:], 0.0)
        htile = None  # per-step pool tile (double-buffered, avoids WAR on hseq store)
        tall = sbuf.tile([2, 1200], F32, name=f"tall{d}", tag=f"scan_tall{d}")
        u1 = sbuf.tile([2, H], F32, name=f"u1{d}", tag=f"scan_u1{d}")
        sf = sbuf.tile([2, H], F32, name=f"sf{d}", tag=f"scan_sf{d}")
        mm = sbuf.tile([2, H], F32, name=f"mm{d}", tag=f"scan_m{d}")
        thc = sbuf.tile([2, H], F32, name=f"thc{d}", tag=f"scan_thc{d}")
        chains.append((d, xp, whh_rhs, hseq, lhsT, ctile, htile, tall, u1, sf, mm, thc))

    for t in range(L):
        # phase 1: stage DMAs + all matmuls for both chains (keeps PE dense:
        # chain b's MMs run while chain f's ACT/DVE work drains)
        gates_by_d = {}
        for (d, xp, whh_rhs, hseq, lhsT, ctile, htile, tall, u1, sf, mm, thc) in chains:
            tt = t if d == "f" else L - 1 - t
            stage = stage_pool.tile([2, GP], F32R, tag=f"stage{d}")
            if not ABLATE.get("no_xp"):
                for j in range(2):
                    nc.sync.dma_start(out=stage[j:j + 1, :],
                                      in_=xp[tt:tt + 1, j * GP:(j + 1) * GP].bitcast(F32R))
            gates = psum_g.tile([2, GP], F32, space="PSUM", tag=f"gates{d}")
            gates_by_d[d] = gates
            for kc in range(3):
                for (n0, nn) in NSL:
                    nc.tensor.matmul(
                        out=gates[:, n0:n0 + nn],
                        lhsT=lhsT[:, 2 * kc:2 * kc + 2],
                        rhs=whh_rhs[:, GP * kc + n0:GP * kc + n0 + nn],
                        start=(kc == 0), stop=False,
                    )
            if not ABLATE.get("no_xp"):
                for si_, (n0, nn) in enumerate(NSL):
                    nc.tensor.matmul(
                        out=gates[:, n0:n0 + nn],
                        lhsT=i2r[:2, :2],
                        rhs=stage[:, n0:n0 + nn],
                        start=False, stop=(si_ == len(NSL) - 1),
                    )
            else:
                for si_, (n0, nn) in enumerate(NSL):
                    nc.tensor.matmul(
                        out=gates[:, n0:n0 + nn], lhsT=lhsT[:, 0:2],
                        rhs=whh_rhs[:, n0:n0 + nn], start=False,
                        stop=(si_ == len(NSL) - 1))
        # phase 2: cell updates, sub-op interleaved so neither chain's small
        # ACT op blocks the other chain's big tanh in the in-order ACT queue
        live = [c for c in chains]
        h_by_d = {}
        for (d, xp, whh_rhs, hseq, lhsT, ctile, htile, tall, u1, sf, mm, thc) in live:
            gates = gates_by_d[d]
            htile = h_by_d[d] = stage_pool.tile([2, H], F32R, name=f"H{d}", tag=f"scan_H{d}")
            if ABLATE.get("no_cell"):
                nc.scalar.activation(out=htile[:].bitcast(F32), in_=gates[:, 0:300],
                                     func=mybir.ActivationFunctionType.Tanh, scale=0.5)
            else:
                nc.scalar.activation(out=tall[:], in_=gates[:, 0:1200],
                                     func=mybir.ActivationFunctionType.Tanh, scale=0.5)
        if not ABLATE.get("no_cell"):
            for (d, xp, whh_rhs, hseq, lhsT, ctile, htile, tall, u1, sf, mm, thc) in live:
                nc.vector.scalar_tensor_tensor(out=u1[:], in0=tall[:, 0:300], scalar=1.0,
                                               in1=tall[:, 900:1200],
                                               op0=mybir.AluOpType.add, op1=mybir.AluOpType.mult)
                nc.vector.tensor_scalar(out=sf[:], in0=tall[:, 300:600], scalar1=0.5, scalar2=0.5,
                                        op0=mybir.AluOpType.mult, op1=mybir.AluOpType.add)
            for (d, xp, whh_rhs, hseq, lhsT, ctile, htile, tall, u1, sf, mm, thc) in live:
                nc.vector.tensor_tensor(out=mm[:], in0=sf[:], in1=ctile[:], op=mybir.AluOpType.mult)
                nc.vector.tensor_tensor(out=ctile[:], in0=mm[:], in1=u1[:], op=mybir.AluOpType.add)
            for (d, xp, whh_rhs, hseq, lhsT, ctile, htile, tall, u1, sf, mm, thc) in live:
                nc.scalar.activation(out=thc[:], in_=ctile[:],
                                     func=mybir.ActivationFunctionType.Tanh, scale=0.5)
            for (d, xp, whh_rhs, hseq, lhsT, ctile, htile, tall, u1, sf, mm, thc) in live:
                nc.vector.scalar_tensor_tensor(out=h_by_d[d][:], in0=tall[:, 600:900],
                                               scalar=1.0, in1=thc[:],
                                               op0=mybir.AluOpType.add, op1=mybir.AluOpType.mult)
        # phase 3: H out + transposes for both chains
        for (d, xp, whh_rhs, hseq, lhsT, ctile, htile, tall, u1, sf, mm, thc) in chains:
            tt = t if d == "f" else L - 1 - t
            htile = h_by_d[d]
            nc.gpsimd.dma_start(out=hseq[:, tt, :], in_=htile[:].bitcast(F32))
            if not ABLATE.get("no_transp"):
                for kc, (c0, cw) in enumerate([(0, 128), (128, 128), (256, 44)]):
                    tp = psum_t.tile([128, 2], F32, space="PSUM", tag="tp")
                    nc.tensor.transpose(out=tp[:cw, :], in_=htile[:, c0:c0 + cw].bitcast(F32),
                                        identity=ident[:2, :2])
                    if kc == 0:
                        nc.scalar.copy(out=lhsT[0:cw, 2 * kc:2 * kc + 2], in_=tp[0:cw, :])
                    else:
                        nc.vector.tensor_copy(out=lhsT[0:cw, 2 * kc:2 * kc + 2], in_=tp[0:cw, :])


def _build_xT_from_hseq(nc, tc, hf_t, hb_t, xT, ident, psum_x, ones_t):
    """hf_t [128,304] (Hf at 0:300), hb_t [128,384] (Hb at cols 44:344) ->
    xT [128, 5*128] f32r: rows = features [Hf 0:300 | Hb 300:600 | ones 600 | pad]."""
    pieces = [
        (hf_t[:, 0:128], 0, 0, 128),
        (hf_t[:, 128:256], 1, 0, 128),
        (hf_t[:, 256:300], 2, 0, 44),
        (hb_t[:, 0:128], 2, 64, 128),      # Hb[0:64] -> feature rows 320:384
        (hb_t[:, 128:256], 3, 0, 128),     # Hb[64:192]
        (hb_t[:, 256:384], 4, 0, 128),     # Hb[192:300]+pad
    ]
    for (src, chunk, r0, r1) in pieces:
        tp = psum_x.tile([128, 128], F32, space="PSUM", tag="tp")
        nc.tensor.transpose(out=tp[:src.shape[1], :], in_=src, identity=ident[:])  # [128,*] full K
        nc.vector.tensor_copy(out=xT[r0:r1, ts(chunk, 128)], in_=tp[r0:r1, :])
    # ones row at feature 620 = chunk 4 row 108 (DMA: engines can't address base 108)
    nc.sync.dma_start(out=xT[108:109, ts(4, 128)], in_=ones_t[:, 0:128].bitcast(F32R))


def _proj(nc, xT, w_rhs, xp_out, col0, kcs, psum_p, sbuf_unused):
    """xp_out[:, col0:col0+GP] = (xT.T @ w_rhs) ; xT [128, kcs*128] f32r,
    w_rhs [128, kcs*GP] f32r, out token-major [128, GP] fp32."""
    pt = psum_p.tile([128, GP], F32, space="PSUM", tag="gatesf")
    for (n0, nn) in NSL:
        for kc in range(kcs):
            nc.tensor.matmul(
                out=pt[:, n0:n0 + nn],
                lhsT=xT[:, ts(kc, 128)],
                rhs=w_rhs[:, GP * kc + n0:GP * kc + n0 + nn],
                start=(kc == 0), stop=(kc == kcs - 1),
            )
    nc.vector.tensor_copy(out=xp_out[:, col0:col0 + GP], in_=pt[:])


_PROGRAM_CACHE = {}


def _build_program():
    if "nc" in _PROGRAM_CACHE:
        return _PROGRAM_CACHE["nc"]
    nc = bass.Bass()
    dp = nc.declare_dram_parameter
    # ---- inputs ----
    emb_word = dp("emb_word", [50000, 300], F32, isOutput=False)
    emb_fixed = dp("emb_fixed", [50000, 300], F32, isOutput=False)
    emb_pos = dp("emb_pos", [60, 16], F32, isOutput=False)
    emb_lemma = dp("emb_lemma", [5000, 300], F32, isOutput=False)
    sent_i = dp("sent_i", [BL, 128], I32, isOutput=False)
    psent_i = dp("psent_i", [BL, 128], I32, isOutput=False)
    pos_i = dp("pos_i", [BL, 128], I32, isOutput=False)
    lemma_i = dp("lemma_i", [BL, 128], I32, isOutput=False)
    region = dp("region", [BL, 128], F32, isOutput=False)
    i4m = dp("i4m", [4, 34], F32, isOutput=False)
    onesrow = dp("onesrow", [1, 128], F32, isOutput=False)
    wih = {}
    for l in range(1, 5):
        kc = 8 if l == 1 else 5
        for d in "fb":
            wih[(l, d)] = dp(f"wih{l}{d}", [128, kc * GP], F32, isOutput=False)
    whh = {(l, d): dp(f"whh{l}{d}", [128, 3 * GP], F32, isOutput=False)
           for l in range(1, 5) for d in "fb"}
    # ---- outputs: the four BiLSTM layer sequences we need downstream ----
    hseq = {}
    for l, n in [(2, "bfe"), (4, "h3")]:
        for d in "fb":
            hseq[(l, d)] = dp(f"hseq_{n}_{d}", [BL, L, H], F32, isOutput=True)
    for l in [1, 3]:
        for d in "fb":
            hseq[(l, d)] = nc.dram_tensor(f"hseq_l{l}_{d}", [BL, L, H], F32)

    with tile.TileContext(nc) as tc:
        import contextlib
        ctx = contextlib.ExitStack()
        with ctx:
            sbuf = ctx.enter_context(tc.tile_pool(name="sbuf", bufs=1))
            psum_g = ctx.enter_context(tc.tile_pool(name="psg", bufs=1, space="PSUM"))
            psum_t = ctx.enter_context(tc.tile_pool(name="pst", bufs=2, space="PSUM"))
            stage_pool = ctx.enter_context(tc.tile_pool(name="stage", bufs=4))
            wpool = ctx.enter_context(tc.tile_pool(name="wpool", bufs=1))

            ident = sbuf.tile([128, 128], F32, tag="ident")
            make_identity(nc, ident[:])
            ones_t = sbuf.tile([1, 128], F32, tag="ones_t")
            nc.sync.dma_start(out=ones_t[:], in_=onesrow[:])
            identr = sbuf.tile([128, 128], F32R, tag="identr")
            nc.vector.tensor_copy(out=identr[:], in_=ident[:])

            # ---- embeddings gather -> x [128, 1024] per b; transpose -> x1T ----
            x1T = [sbuf.tile([128, 8 * 128], F32R, name=f"x1T{b}", tag=f"x1T{b}") for b in range(BL)]
            for b in range(BL):
                xt = sbuf.tile([128, K1], F32, tag="xgath")
                nc.gpsimd.memset(xt[:], 0.0)
                for (idx_t, table, c0, w) in [
                    (sent_i, emb_word, 0, 300),
                    (psent_i, emb_fixed, 300, 300),
                    (pos_i, emb_pos, 600, 16),
                    (lemma_i, emb_lemma, 616, 300),
                ]:
                    it = sbuf.tile([128, 1], I32, tag="idx")
                    nc.sync.dma_start(out=it[:, 0:1], in_=idx_t[b, :, None])
                    nc.gpsimd.indirect_dma_start(
                        out=xt[:, c0:c0 + w], out_offset=None, in_=table[:],
                        in_offset=bass.IndirectOffsetOnAxis(ap=it[:, :1], axis=0),
                    )
                nc.sync.dma_start(out=xt[:, 916:917], in_=region[b, :, None])
                nc.gpsimd.memset(xt[:, 917:918], 1.0)
                for kc in range(8):
                    tp = psum_t.tile([128, 128], F32, space="PSUM", tag="tp")
                    nc.tensor.transpose(out=tp[:], in_=xt[:, ts(kc, 128)], identity=ident[:])
                    nc.vector.tensor_copy(out=x1T[b][:, ts(kc, 128)], in_=tp[:])

            # ---- per-layer: projections then scan ----
            hf_t = [sbuf.tile([128, 304], F32, name=f"hf{b}", tag=f"hf{b}") for b in range(BL)]
            hb_t = [sbuf.tile([128, 384], F32, name=f"hb{b}", tag=f"hb{b}") for b in range(BL)]
            x2T = [sbuf.tile([128, 5 * 128], F32R, name=f"x2T{b}", tag=f"x2T{b}") for b in range(BL)]
            for b in range(BL):
                nc.gpsimd.memset(hb_t[b][:], 0.0)
                nc.gpsimd.memset(x2T[b][:].bitcast(F32), 0.0)

            for l in range(1, 5):
                kc = 8 if l == 1 else 5
                xp_f = sbuf.tile([128, 2 * GP], F32, tag="xp_f")
                xp_b = sbuf.tile([128, 2 * GP], F32, tag="xp_b")
                wtile = wpool.tile([128, kc * GP], F32R, tag="wtile")
                for d, xp in (("f", xp_f), ("b", xp_b)):
                    nc.sync.dma_start(out=wtile[:], in_=wih[(l, d)][:].bitcast(F32R))
                    for b in range(BL):
                        if l == 1:
                            xT = x1T[b]
                        else:
                            xT = x2T[b]
                        _proj(nc, xT, wtile, xp, b * GP, kc, psum_g, sbuf)
                whh_rhs_f = wpool.tile([128, 3 * GP], F32R, tag="whh_rhs_f")
                nc.sync.dma_start(out=whh_rhs_f[:], in_=whh[(l, "f")][:].bitcast(F32R))
                whh_rhs_b = wpool.tile([128, 3 * GP], F32R, tag="whh_rhs_b")
                nc.sync.dma_start(out=whh_rhs_b[:], in_=whh[(l, "b")][:].bitcast(F32R))
                if not ABLATE.get("no_scan"):
                    _layer_scan(nc, tc, ctx, f"l{l}", xp_f, xp_b, whh_rhs_f, whh_rhs_b,
                                identr, ident,
                                hseq[(l, "f")], hseq[(l, "b")], sbuf, psum_g, psum_t, stage_pool)
                if l < 4:
                    for b in range(BL):
                        nc.sync.dma_start(out=hf_t[b][:, 0:300], in_=hseq[(l, "f")][b, :, :])
                        nc.sync.dma_start(out=hb_t[b][:, HB_OFF:HB_OFF + 300],  # Hb at cols 64:364
                                          in_=hseq[(l, "b")][b, :, :])
                        _build_xT_from_hseq(nc, tc, hf_t[b], hb_t[b], x2T[b], ident, psum_t, ones_t)

    _split_all_waits(nc)
    _PROGRAM_CACHE["nc"] = nc
    return nc


def _prep_lstm_weights(params):
    """Reorder gates i,f,g,o -> i,f,o,g; double g rows (tanh(0.5*2x)=tanh x);
    scale input weights of consumers of H=2h by 0.5; fold bias as ones-feature."""
    def reorder(w):
        i, f, g, o = np.split(w, 4, axis=0)
        return np.concatenate([i, f, o, 2.0 * g], axis=0)

    out = {}
    layers = list(params['lstm_share']) + list(params['lstm_srl'])
    for li, dirs in enumerate(layers, start=1):
        kc = 8 if li == 1 else 5
        for di, dname in enumerate("fb"):
            p = dirs[di]
            wih = reorder(np.asarray(p['wih'], np.float32))      # [1200, din]
            bb_ = reorder(np.asarray(p['b'], np.float32)[:, None])[:, 0]
            whh_ = reorder(np.asarray(p['whh'], np.float32))     # [1200, 300]
            din = wih.shape[1]
            if li > 1:
                wih = wih * 0.5           # input is H=2h
            whh_ = whh_ * 0.5             # recurrent input is H=2h
            wr = np.zeros((kc * 128, GP), np.float32)
            if li == 1:
                wr[:din, :1200] = wih.T
                wr[917, :1200] = bb_      # ones feature at col 917
            else:
                wr[0:300, :1200] = wih.T[0:300]      # Hf features
                wr[320:620, :1200] = wih.T[300:600]  # Hb features
                wr[620, :1200] = bb_                 # ones feature
            out[f"wih{li}{dname}"] = np.ascontiguousarray(
                wr.reshape(kc, 128, GP).transpose(1, 0, 2).reshape(128, kc * GP))
            whh_blk = np.zeros((3 * 128, GP), np.float32)
            whh_blk[0:300, :1200] = whh_.T
            out[f"whh{li}{dname}"] = np.ascontiguousarray(
                whh_blk.reshape(3, 128, GP).transpose(1, 0, 2).reshape(128, 3 * GP))
    return out


def _host_post(bf_e, h3, inputs, params):
    """Exact fp32 post-LSTM math on host. bf_e/h3: [B, L, 600] true scale."""
    p = params
    _BIG = 10.0 ** 6.0
    dep_heads = np.asarray(inputs['dep_heads'])
    idx = np.clip(dep_heads - 1, 0, L - 1)
    gathered = np.take_along_axis(bf_e, idx[:, :, None], axis=1)
    concat_embeds = np.where((dep_heads > 0)[:, :, None], gathered, 0.0).astype(np.float32)

    def lin(x, pp):
        return x @ np.asarray(pp['w'], np.float32).T + np.asarray(pp['b'], np.float32)

    dep_tag_space = lin(np.tanh(lin(bf_e, p['h2t_M']) + lin(concat_embeds, p['h2t_H'])),
                        p['mlp']).reshape(B * L, -1)
    pred = h3[np.arange(B), np.asarray(inputs['target_idx_in'])]
    hs = np.concatenate([h3, np.broadcast_to(pred[:, None, :], h3.shape)], axis=-1)
    role = np.concatenate([np.asarray(p['role_emb'], np.float32)[np.asarray(inputs['local_roles_voc'])],
                           np.asarray(p['frame_emb'], np.float32)[np.asarray(inputs['frames'])]],
                          axis=-1)
    mapped = np.maximum(lin(role, p['role_map']), 0.0)
    tag_space = np.einsum('blh,brh->blr', hs, mapped)
    sub = (np.asarray(inputs['local_roles_mask'], np.float32) - 1.0) * _BIG
    tag_space = tag_space + sub[:, None, :]
    tag2 = tag_space.reshape(B * L, -1).astype(np.float32)

    def logsoftmax(x):
        m = x.max(axis=1, keepdims=True)
        e = np.exp(x - m)
        return (x - m) - np.log(e.sum(axis=1, keepdims=True))

    SRLprobs = np.exp(logsoftmax(tag2))

    def ce_ignore0(logits, tgt):
        logp = logsoftmax(logits)
        nll = -logp[np.arange(logits.shape[0]), tgt]
        m = (tgt != 0).astype(np.float32)
        return (nll * m).sum() / max(m.sum(), 1.0)

    dep_labels = np.argmax(dep_tag_space, axis=1)
    gold = np.asarray(inputs['dep_tags']).reshape(-1)
    all_l = np.float32((gold != 0).sum())
    wrong_l = np.float32(((dep_labels != gold) & (gold != 0)).sum())
    SRLloss = np.float32(ce_ignore0(tag2, np.asarray(inputs['targets']).reshape(-1)))
    DEPloss = np.float32(ce_ignore0(dep_tag_space, gold))
    return SRLloss, DEPloss, np.float32(SRLloss + DEPloss), SRLprobs.astype(np.float32), wrong_l, all_l


def kernel(**inputs):
    params = inputs['params']
    nc = _build_program()
    wmap = _prep_lstm_weights(params)

    def np32(x):
        return np.ascontiguousarray(np.asarray(x, np.float32))

    tables = {
        "emb_word": np32(params['word_emb']),
        "emb_fixed": np32(params['word_fixed']),
        "emb_pos": np32(params['pos_emb']),
        "emb_lemma": np32(params['p_lemma_emb']),
    }
    in_maps = []
    for c in range(NC):
        sl = slice(c * BL, (c + 1) * BL)
        m = dict(tables)
        m.update({k: v for k, v in wmap.items()})
        m["sent_i"] = np.ascontiguousarray(np.asarray(inputs['sentence'][sl], np.int32))
        m["psent_i"] = np.ascontiguousarray(np.asarray(inputs['p_sentence'][sl], np.int32))
        m["pos_i"] = np.ascontiguousarray(np.asarray(inputs['pos_tags'][sl], np.int32))
        m["lemma_i"] = np.ascontiguousarray(np.asarray(inputs['sent_pred_lemmas_idx'][sl], np.int32))
        m["region"] = np32(inputs['region_marks'][sl])
        i4v = np.zeros((4, 34), np.float32)
        for j, col in enumerate([0, 1, 32, 33]):
            i4v[j, col] = 1.0
        m["i4m"] = i4v
        m["onesrow"] = np.ones((1, 128), np.float32)
        in_maps.append(m)

    import os
    trace = bool(os.environ.get("BASS_TRACE"))
    res = run_bass_kernel_spmd(nc, in_maps, core_ids=list(range(NC)), trace=trace)
    globals()["LAST_RESULTS"] = res
    bf_e = np.zeros((B, L, 2 * H), np.float32)
    h3 = np.zeros((B, L, 2 * H), np.float32)
    for c in range(NC):
        r = res.results[c]
        sl = slice(c * BL, (c + 1) * BL)
        bf_e[sl, :, 0:H] = r["hseq_bfe_f"] * 0.5   # H=2h -> true h
        bf_e[sl, :, H:2 * H] = r["hseq_bfe_b"] * 0.5
        h3[sl, :, 0:H] = r["hseq_h3_f"] * 0.5
        h3[sl, :, H:2 * H] = r["hseq_h3_b"] * 0.5
    return _host_post(bf_e, h3, inputs, params)
